# revision 6
# baseline (speedup 1.0000x reference)
"""Multi-head causal self-attention on 8 Trainium2 NeuronCores.

Problem: B=4, T=2048, C=1024, H=16 heads (DH=64), causal mask, fp32 I/O.

Sharding: core i handles batch b=i//2 and head-group g=i%2 (8 heads).
Per-core compute (bf16 matmuls, fp32 accumulation):
  - QKV projection for its 8 heads:  qT/kT in [d', t] layout, V in [t, d']
    layout with an appended ones-column (gives softmax row-sums for free
    during the AV matmul).
  - Causal attention: S^T = kT^T @ qT per (128-key, 512-query) block,
    exp on ScalarE straight out of PSUM (batched over 2 banks), triangular
    masks applied multiplicatively on VectorE for diagonal blocks, then
    O^T (+row-sums) accumulated in PSUM via the AV matmul.
  - Normalization by reciprocal row-sums (broadcast via a DRAM bounce).
  - Output projection partial product, bias on g=0 cores only, then a
    pair-wise ReduceScatter sums the two head-groups of each batch and
    leaves each core with half the rows of its batch's output.
Host assembles the full [4, 2048, 1024] output from the 8 shards.
"""
import sys

if "/opt/trn_rl_repo" not in sys.path:
    sys.path.insert(0, "/opt/trn_rl_repo")

import numpy as np
import ml_dtypes

import concourse.mybir as mybir
import concourse.tile as tile
from concourse import bacc
from concourse.bass_utils import run_bass_kernel_spmd

B, T, C, H = 4, 2048, 1024, 16
DH = C // H              # 64
HL = H // 2              # 8 heads per core
CL = HL * DH             # 512 local channels
THALF = T // 2           # 1024 rows of output per core after ReduceScatter

bf16 = mybir.dt.bfloat16
f32 = mybir.dt.float32
BF = ml_dtypes.bfloat16

REPLICA_GROUPS = [[0, 1], [2, 3], [4, 5], [6, 7]]


def build_body(nc, tc, ext, pools, rep_tag=""):
    """Emit one full forward pass. `ext` holds external APs, `pools` the
    shared tile pools (so repeated bodies reuse SBUF/PSUM slots)."""
    sb, big, ps, pt_pool, small, dram = (
        pools["sb"], pools["big"], pools["ps"], pools["pt"], pools["small"],
        pools["dram"],
    )
    Exp = mybir.ActivationFunctionType.Exp
    mul = mybir.AluOpType.mult

    xs = ext["xs"]; wq_s = ext["wq_s"]; wk_s = ext["wk_s"]; wv_s = ext["wv_s"]
    wp_s = ext["wp_s"]; bqs = ext["bqs"]; bks = ext["bks"]; bvs = ext["bvs"]
    pbs = ext["pbs"]; tris = ext["tris"]
    out_ext = ext["out"]

    # ---- working tiles for this pass ----
    # per-t-chunk tiles so attention(qt) only depends on the chunks it reads
    vts = ext["vts"]
    qts, kts = [], []
    for n in range(4):
        qt_n = big.tile([128, 4, 512], bf16, tag=f"qT{n}")
        kt_n = big.tile([128, 4, 512], bf16, tag=f"kT{n}")
        qts.append(qt_n)
        kts.append(kt_n)
    pdt = bf16 if ext.get("proj_bf16") else f32
    proj_dram = dram.tile([T, C], pdt, tag="proj")
    rs_out = dram.tile([THALF, C], pdt, tag="rs")

    def emit_qkv_chunk(n):
        # qT/kT: out[d' 128, t 512] = sum_cc w[:, cc, d'-slab].T @ xT[:, cc, t]
        for m in range(4):              # d' slabs of 128
            for w_s, dst, bias in ((wq_s, qts[n], bqs), (wk_s, kts[n], bks)):
                ps_t = ps.tile([128, 512], f32, tag="P")
                for cc in range(8):
                    nc.tensor.matmul(
                        ps_t[:],
                        lhsT=w_s[:, cc, m * 128:(m + 1) * 128],
                        rhs=xs[:, cc, n * 512:(n + 1) * 512],
                        start=(cc == 0), stop=(cc == 7),
                    )
                nc.vector.tensor_tensor(
                    dst[:, m, :], ps_t[:],
                    bias[:, m:m + 1].to_broadcast([128, 512]),
                    mybir.AluOpType.add)
        # V: out[t 128, d' 512] = sum_cc xT[:, cc, tt].T @ wv[:, cc, :]
        for tl in range(4):
            tt = 4 * n + tl
            ps_t = ps.tile([128, 512], f32, tag="P")
            for cc in range(8):
                nc.tensor.matmul(
                    ps_t[:],
                    lhsT=xs[:, cc, tt * 128:(tt + 1) * 128],
                    rhs=wv_s[:, cc, :],
                    start=(cc == 0), stop=(cc == 7),
                )
            nc.vector.tensor_tensor(
                vts[n][:, :, tl, 0:DH],
                ps_t[:].rearrange("p (h d) -> p h d", h=HL),
                bvs[:].rearrange("p (h d) -> p h d", h=HL),
                mybir.AluOpType.add,
            )

    if "attn" in ext.get("skip", ()):
        # sink qkv outputs so DCE keeps the QKV phase
        snk = ext["snk"]
        for n in range(4):
            emit_qkv_chunk(n)
            nc.sync.dma_start(out=snk[0:128, n * 2048:(n + 1) * 2048],
                              in_=qts[n][:].rearrange("p a b -> p (a b)"))
        return

    def emit_proj(qt, attnT):
        # output projection for this q-tile
        for tt in range(4):
            t0 = qt * 512 + tt * 128
            for cn in range(2):
                ps_P = ps.tile([128, 512], f32, tag="P", name="P")
                for j in range(4):
                    nc.tensor.matmul(
                        ps_P[:],
                        lhsT=attnT[:, j, tt * 128:(tt + 1) * 128],
                        rhs=wp_s[:, j, cn * 512:(cn + 1) * 512],
                        start=(j == 0), stop=(j == 3),
                    )
                ot = small.tile([128, 512], pdt, tag="ot", name="ot")
                nc.vector.tensor_tensor(
                    ot[:], ps_P[:], pbs[:, cn * 512:(cn + 1) * 512],
                    mybir.AluOpType.add)
                nc.sync.dma_start(
                    out=proj_dram[t0:t0 + 128, cn * 512:(cn + 1) * 512],
                    in_=ot[:])

        # pairwise ReduceScatter + output DMA for this q-tile's rows
        if ext.get("no_rs"):
            nc.sync.dma_start(
                out=rs_out[qt * 256:(qt + 1) * 256, :],
                in_=proj_dram[qt * 512:qt * 512 + 256, :])
        elif ext.get("single_core"):
            nc.gpsimd.dma_start(
                out=out_ext[qt * 256:(qt + 1) * 256, :],
                in_=proj_dram[qt * 512:qt * 512 + 256, :])
        else:
            nc.gpsimd.collective_compute(
                "ReduceScatter",
                mybir.AluOpType.add,
                replica_groups=REPLICA_GROUPS,
                ins=[proj_dram[qt * 512:(qt + 1) * 512, :].opt()],
                outs=[rs_out[qt * 256:(qt + 1) * 256, :].opt()],
            )
            if ext.get("proj_bf16"):
                for hh in range(2):
                    r0 = qt * 256 + hh * 128
                    stg = small.tile([128, C], bf16, tag="stg", name="stg")
                    nc.sync.dma_start(out=stg[:], in_=rs_out[r0:r0 + 128, :])
                    stf = small.tile([128, C], f32, tag="stf", name="stf")
                    nc.vector.tensor_copy(stf[:], stg[:])
                    nc.sync.dma_start(out=out_ext[r0:r0 + 128, :], in_=stf[:])
            else:
                nc.sync.dma_start(
                    out=out_ext[qt * 256:(qt + 1) * 256, :],
                    in_=rs_out[qt * 256:(qt + 1) * 256, :])

    dk = ext.get("defer_k", 1)
    # ---- QKV production, then attention ----
    if ext.get("ldwshare"):
        # Q/K reordered so consecutive MMs share an identical lhsT AP:
        # for each (weight, slab, cc) load, stream two n-chunks into the
        # two P-tag PSUM slots. If the toolchain elides repeated weight
        # loads, half the Q/K LDWs disappear.
        for np2 in range(2):                    # n-chunk pairs (0,1), (2,3)
            n0, n1 = 2 * np2, 2 * np2 + 1
            for m in range(4):
                for w_s, dsts, bias in ((wq_s, qts, bqs), (wk_s, kts, bks)):
                    ps_a = ps.tile([128, 512], f32, tag="P", name="Pa")
                    ps_b = ps.tile([128, 512], f32, tag="P", name="Pb")
                    for cc in range(8):
                        lhs = w_s[:, cc, m * 128:(m + 1) * 128]
                        nc.tensor.matmul(
                            ps_a[:], lhsT=lhs,
                            rhs=xs[:, cc, n0 * 512:(n0 + 1) * 512],
                            start=(cc == 0), stop=(cc == 7))
                        nc.tensor.matmul(
                            ps_b[:], lhsT=lhs,
                            rhs=xs[:, cc, n1 * 512:(n1 + 1) * 512],
                            start=(cc == 0), stop=(cc == 7))
                    for pst, n in ((ps_a, n0), (ps_b, n1)):
                        nc.vector.tensor_tensor(
                            dsts[n][:, m, :], pst[:],
                            bias[:, m:m + 1].to_broadcast([128, 512]),
                            mybir.AluOpType.add)
        for n in range(4):                      # V chains unchanged
            for tl in range(4):
                tt = 4 * n + tl
                ps_t = ps.tile([128, 512], f32, tag="P", name="Pv2")
                for cc in range(8):
                    nc.tensor.matmul(
                        ps_t[:],
                        lhsT=xs[:, cc, tt * 128:(tt + 1) * 128],
                        rhs=wv_s[:, cc, :],
                        start=(cc == 0), stop=(cc == 7),
                    )
                nc.vector.tensor_tensor(
                    vts[n][:, :, tl, 0:DH],
                    ps_t[:].rearrange("p (h d) -> p h d", h=HL),
                    bvs[:].rearrange("p (h d) -> p h d", h=HL),
                    mybir.AluOpType.add,
                )
    elif ext.get("qk1024"):
        # Q/K with N=1024 moving operand: halves the Q/K matmul count.
        # Chains use the (idle during QKV) "S" tag's [128,1024] PSUM slots.
        for m in range(4):
            for w_s, dsts, bias in ((wq_s, qts, bqs), (wk_s, kts, bks)):
                for np2 in range(2):            # n-chunk pairs (0,1), (2,3)
                    ps_t = ps.tile([128, 1024], f32, tag="S", name="Pqk")
                    for cc in range(8):
                        nc.tensor.matmul(
                            ps_t[:],
                            lhsT=w_s[:, cc, m * 128:(m + 1) * 128],
                            rhs=xs[:, cc, np2 * 1024:(np2 + 1) * 1024],
                            start=(cc == 0), stop=(cc == 7),
                        )
                    for e in range(2):
                        n = 2 * np2 + e
                        nc.vector.tensor_tensor(
                            dsts[n][:, m, :], ps_t[:, e * 512:(e + 1) * 512],
                            bias[:, m:m + 1].to_broadcast([128, 512]),
                            mybir.AluOpType.add)
        for n in range(4):                      # V chains unchanged
            for tl in range(4):
                tt = 4 * n + tl
                ps_t = ps.tile([128, 512], f32, tag="P", name="Pv")
                for cc in range(8):
                    nc.tensor.matmul(
                        ps_t[:],
                        lhsT=xs[:, cc, tt * 128:(tt + 1) * 128],
                        rhs=wv_s[:, cc, :],
                        start=(cc == 0), stop=(cc == 7),
                    )
                nc.vector.tensor_tensor(
                    vts[n][:, :, tl, 0:DH],
                    ps_t[:].rearrange("p (h d) -> p h d", h=HL),
                    bvs[:].rearrange("p (h d) -> p h d", h=HL),
                    mybir.AluOpType.add,
                )
    else:
        for n in range(4):
            emit_qkv_chunk(n)
    attnTs = {}
    for qt in range(4):
        attnT = sb.tile([128, 4, 512], bf16, tag="attnT", name="attnT")
        kmax = 4 * (qt + 1)
        for h in range(HL):
            j, half = h // 2, (h % 2) * 64
            ps_O = ps.tile([128, 512], f32, tag="O")
            for p in range(kmax // 2):
                kc0 = 2 * p
                # widths: diagonal chunks only need the causally-valid
                # query suffix (d = kc - 4*qt -> width 512 - 128*d)
                ws = []
                for e in range(2):
                    d = (kc0 + e) - 4 * qt
                    ws.append(512 if d < 0 else 512 - 128 * d)
                # pack the two S blocks contiguously: e=0 at [0:w0],
                # e=1 at [w0:w0+w1] (no PSUM gap for the exp to read)
                offs = [0, ws[0]]
                ps_S = ps.tile([128, 1024], f32, tag="S")
                for e in range(2):
                    kc, w = kc0 + e, ws[e]
                    nc.tensor.matmul(
                        ps_S[:, offs[e]:offs[e] + w],
                        lhsT=kts[kc // 4][half:half + 64, j,
                                          (kc % 4) * 128:(kc % 4 + 1) * 128],
                        rhs=qts[qt][half:half + 64, j, 512 - w:512],
                        start=True, stop=True,
                    )
                pt = pt_pool.tile([128, 1024], bf16, tag="pt")
                espan = ws[0] + ws[1]
                nc.scalar.activation(pt[:, 0:espan], ps_S[:, 0:espan], Exp,
                                     scale=DH ** -0.5)
                if ext.get("narrowmask"):
                    # only the first 128 cols of a diagonal chunk straddle
                    # the causal boundary; mask just those
                    for e in range(2):
                        if (kc0 + e) >= 4 * qt:
                            nc.vector.tensor_tensor(
                                pt[:, offs[e]:offs[e] + 128],
                                pt[:, offs[e]:offs[e] + 128],
                                tris[:, 0:128], mul)
                elif kc0 >= 4 * qt:      # diagonal pair -> causal mask
                    dp = (kc0 - 4 * qt) // 2
                    nc.vector.tensor_tensor(
                        pt[:, 0:espan], pt[:, 0:espan],
                        tris[:, dp * 1024:dp * 1024 + espan], mul)
                for e in range(2):
                    kc, w = kc0 + e, ws[e]
                    nc.tensor.matmul(
                        ps_O[:, 512 - w:512],
                        lhsT=vts[kc // 4][:, h, kc % 4, :],
                        rhs=pt[:, offs[e]:offs[e] + w],
                        start=(kc == 0), stop=(kc == kmax - 1),
                    )
            rb = small.tile([64, 512], bf16, tag="rb")
            with nc.allow_low_precision(reason="bf16 recip, bf16 mult"):
                nc.vector.reciprocal(rb[:], ps_O[64:128, :])
            nc.vector.tensor_tensor(attnT[half:half + 64, j, :],
                                    ps_O[0:64, :], rb[:], mul)
        attnTs[qt] = attnT

        if "proj" in ext.get("skip", ()):
            snk = ext["snk"]
            nc.sync.dma_start(out=snk[qt * 128:(qt + 1) * 128, 0:2048],
                              in_=attnT[:].rearrange("p a b -> p (a b)"))
            continue
        if dk:
            if qt >= dk:
                emit_proj(qt - dk, attnTs[qt - dk])
            continue
        emit_proj(qt, attnT)



    if dk and "proj" not in ext.get("skip", ()):
        for r in range(4 - dk, 4):
            emit_proj(r, attnTs[r])


def build_body_v2(nc, tc, ext, pools):
    """Software-pipelined body: head-pair attention units with concurrent
    row-group S matmuls, pair-packed PSUM score tiles (one exp per kc-pair),
    and QKV/proj chains interleaved between attention units as PE filler."""
    sb, big, ps1, ps2, pt_pool, small, dram = (
        pools["sb"], pools["big"], pools["ps1"], pools["ps2"], pools["pt"],
        pools["small"], pools["dram"],
    )
    Exp = mybir.ActivationFunctionType.Exp
    mul = mybir.AluOpType.mult
    add = mybir.AluOpType.add
    scale = DH ** -0.5

    xs = ext["xs"]; wq_s = ext["wq_s"]; wk_s = ext["wk_s"]; wv_s = ext["wv_s"]
    wp_s = ext["wp_s"]; bqs = ext["bqs"]; bks = ext["bks"]; bvs = ext["bvs"]
    pbs = ext["pbs"]; tris = ext["tris"]
    out_ext = ext["out"]
    vts = ext["vts"]

    qts, kts = [], []
    for n in range(4):
        qts.append(big.tile([128, 4, 512], bf16, tag=f"qT{n}", name=f"qT{n}"))
        kts.append(big.tile([128, 4, 512], bf16, tag=f"kT{n}", name=f"kT{n}"))
    pdt = bf16 if ext.get("proj_bf16") else f32
    proj_dram = dram.tile([T, C], pdt, tag="proj", name="proj")
    rs_out = dram.tile([THALF, C], pdt, tag="rs", name="rs")

    def qkv_chunk_units(n):
        units = []
        for m in range(4):
            for w_s, dsts, bias in ((wq_s, qts, bqs), (wk_s, kts, bks)):
                def u(m=m, w_s=w_s, dsts=dsts, bias=bias, n=n):
                    ps_t = ps2.tile([128, 512], f32, tag="P", name="Pqk")
                    for cc in range(8):
                        nc.tensor.matmul(
                            ps_t[:],
                            lhsT=w_s[:, cc, m * 128:(m + 1) * 128],
                            rhs=xs[:, cc, n * 512:(n + 1) * 512],
                            start=(cc == 0), stop=(cc == 7),
                        )
                    nc.vector.tensor_tensor(
                        dsts[n][:, m, :], ps_t[:],
                        bias[:, m:m + 1].to_broadcast([128, 512]), add)
                units.append(u)
        for tl in range(4):
            def u(tl=tl, n=n):
                tt = 4 * n + tl
                ps_t = ps2.tile([128, 512], f32, tag="P", name="Pv")
                for cc in range(8):
                    nc.tensor.matmul(
                        ps_t[:],
                        lhsT=xs[:, cc, tt * 128:(tt + 1) * 128],
                        rhs=wv_s[:, cc, :],
                        start=(cc == 0), stop=(cc == 7),
                    )
                nc.vector.tensor_tensor(
                    vts[n][:, :, tl, 0:DH],
                    ps_t[:].rearrange("p (h d) -> p h d", h=HL),
                    bvs[:].rearrange("p (h d) -> p h d", h=HL),
                    add,
                )
            units.append(u)
        return units

    def proj_units(qt, attnT):
        units = []
        for tt in range(4):
            t0 = qt * 512 + tt * 128
            for cn in range(2):
                def u(tt=tt, cn=cn, t0=t0, attnT=attnT):
                    ps_P = ps2.tile([128, 512], f32, tag="P", name="Pp")
                    for jj in range(4):
                        nc.tensor.matmul(
                            ps_P[:],
                            lhsT=attnT[:, jj, tt * 128:(tt + 1) * 128],
                            rhs=wp_s[:, jj, cn * 512:(cn + 1) * 512],
                            start=(jj == 0), stop=(jj == 3),
                        )
                    ot = small.tile([128, 512], pdt, tag="ot", name="ot")
                    nc.vector.tensor_tensor(
                        ot[:], ps_P[:], pbs[:, cn * 512:(cn + 1) * 512], add)
                    nc.sync.dma_start(
                        out=proj_dram[t0:t0 + 128, cn * 512:(cn + 1) * 512],
                        in_=ot[:])
                units.append(u)
        return units

    def rs_unit(qt):
        def u(qt=qt):
            if ext.get("no_rs"):
                nc.sync.dma_start(
                    out=rs_out[qt * 256:(qt + 1) * 256, :],
                    in_=proj_dram[qt * 512:qt * 512 + 256, :])
            else:
                nc.gpsimd.collective_compute(
                    "ReduceScatter",
                    mybir.AluOpType.add,
                    replica_groups=REPLICA_GROUPS,
                    ins=[proj_dram[qt * 512:(qt + 1) * 512, :].opt()],
                    outs=[rs_out[qt * 256:(qt + 1) * 256, :].opt()],
                )
                if ext.get("proj_bf16"):
                    for hh in range(2):
                        r0 = qt * 256 + hh * 128
                        stg = small.tile([128, C], bf16, tag="stg",
                                         name="stg")
                        nc.sync.dma_start(out=stg[:], in_=rs_out[r0:r0 + 128, :])
                        stf = small.tile([128, C], f32, tag="stf", name="stf")
                        nc.vector.tensor_copy(stf[:], stg[:])
                        nc.sync.dma_start(out=out_ext[r0:r0 + 128, :],
                                          in_=stf[:])
                else:
                    nc.sync.dma_start(
                        out=out_ext[qt * 256:(qt + 1) * 256, :],
                        in_=rs_out[qt * 256:(qt + 1) * 256, :])
        return [u]

    def attn_unit(qt, j, attnT):
        # Per-kc processing with ping-pong score tiles: halves side-by-side
        # in one [128,1024] tile (half0 at [0:w], half1 at [512:512+w]), one
        # exp per kc covering both halves, so exp(kc) overlaps S MMs(kc+1).
        # The two S MMs target row-groups (0,·)/(64,·) and run concurrently.
        kmax = 4 * (qt + 1)
        ps_O0 = ps1.tile([128, 512], f32, tag="O0", name="O0")
        ps_O1 = ps1.tile([128, 512], f32, tag="O1", name="O1")
        for kc in range(kmax):
            d = kc - 4 * qt
            w = 512 if d < 0 else 512 - 128 * d
            ps_S = ps1.tile([128, 1024], f32, tag=f"S{kc % 2}",
                            name="ps_S")
            for half, base in ((0, 0), (64, 512)):
                nc.tensor.matmul(
                    ps_S[:, base:base + w],
                    lhsT=kts[kc // 4][half:half + 64, j,
                                      (kc % 4) * 128:(kc % 4 + 1) * 128],
                    rhs=qts[qt][half:half + 64, j, 512 - w:512],
                    start=True, stop=True,
                )
            pt = pt_pool.tile([128, 1024], bf16, tag="pt", name="pt")
            # single exp over [0:512+w] spans the [w:512] hole when w<512
            # (wasted cols cost less than a second activation's overhead)
            nc.scalar.activation(pt[:, 0:512 + w], ps_S[:, 0:512 + w], Exp,
                                 scale=scale)
            if d >= 0:
                # causal boundary lives in the first 128 cols of the block
                for base in (0, 512):
                    nc.vector.tensor_tensor(
                        pt[:, base:base + 128], pt[:, base:base + 128],
                        tris[:, 0:128], mul)
            for hi, (base, ps_O) in enumerate(((0, ps_O0), (512, ps_O1))):
                nc.tensor.matmul(
                    ps_O[:, 512 - w:512],
                    lhsT=vts[kc // 4][:, 2 * j + hi, kc % 4, :],
                    rhs=pt[:, base:base + w],
                    start=(kc == 0), stop=(kc == kmax - 1),
                )
        for half, ps_O in ((0, ps_O0), (64, ps_O1)):
            rb = small.tile([64, 512], bf16, tag="rb", name="rb")
            with nc.allow_low_precision(reason="bf16 recip, bf16 mult"):
                nc.vector.reciprocal(rb[:], ps_O[64:128, :])
            nc.vector.tensor_tensor(attnT[half:half + 64, j, :],
                                    ps_O[0:64, :], rb[:], mul)

    # ---- emission: prologue, then attention rounds with filler ----
    from collections import deque
    filler = deque()
    for u in qkv_chunk_units(0):
        u()
    filler.extend(qkv_chunk_units(1))
    attnTs = {}
    for qt in range(4):
        attnT = sb.tile([128, 4, 512], bf16, tag="attnT", name="attnT")
        for j in range(4):
            attn_unit(qt, j, attnT)
            npop = (len(filler) + (3 - j)) // (4 - j)
            for _ in range(npop):
                filler.popleft()()
        attnTs[qt] = attnT
        if qt == 0:
            filler.extend(qkv_chunk_units(2))
        elif qt == 1:
            filler.extend(qkv_chunk_units(3))
            filler.extend(proj_units(0, attnTs[0]))
            filler.extend(rs_unit(0))
        elif qt == 2:
            filler.extend(proj_units(1, attnTs[1]))
            filler.extend(rs_unit(1))
            filler.extend(proj_units(2, attnTs[2]))
            filler.extend(rs_unit(2))
    while filler:
        filler.popleft()()
    for u in proj_units(3, attnTs[3]) + rs_unit(3):
        u()


def build_graph(reps=1, single_core=False, no_rs=False, skip=(),
                norm_dma=False, wide_exp=False, loop_n=0,
                pt_bufs=4, sb_bufs=2, small_bufs=3, swap_side=False,
                proj_bf16=True, defer_k=1, qk1024=False, narrowmask=False,
                ldwshare=False, stagger=False, body_reps=1, v2=True):
    nc = bacc.Bacc("TRN2", target_bir_lowering=False, debug=False,
                   num_devices=1 if single_core else 8)
    xT_e = nc.dram_tensor("xT", [C, T], bf16, kind="ExternalInput").ap()
    wq_e = nc.dram_tensor("wq", [C, CL], bf16, kind="ExternalInput").ap()
    wk_e = nc.dram_tensor("wk", [C, CL], bf16, kind="ExternalInput").ap()
    wv_e = nc.dram_tensor("wv", [C, CL], bf16, kind="ExternalInput").ap()
    wp_e = nc.dram_tensor("wp", [CL, C], bf16, kind="ExternalInput").ap()
    bq_e = nc.dram_tensor("bq", [128, 4], f32, kind="ExternalInput").ap()
    bk_e = nc.dram_tensor("bk", [128, 4], f32, kind="ExternalInput").ap()
    bv_e = nc.dram_tensor("bv", [1, CL], f32, kind="ExternalInput").ap()
    pb_e = nc.dram_tensor("pb", [1, C], f32, kind="ExternalInput").ap()
    tri_e = nc.dram_tensor("tri", [128, 2048], bf16, kind="ExternalInput").ap()
    out_e = nc.dram_tensor("out", [THALF, C], f32, kind="ExternalOutput").ap()
    snk_e = (nc.dram_tensor("snk", [512, 8320], bf16, kind="ExternalOutput").ap()
             if skip else None)

    if v2:
        sb_bufs = max(sb_bufs, 4)   # attnT read by proj up to 2 rounds later
        pt_bufs = min(pt_bufs, 3)   # [128,2048] bf16 pair tiles
    with tile.TileContext(nc) as tc:
        if swap_side:
            tc.swap_default_side()
        with (
            tc.tile_pool(name="const", bufs=1) as const,
            tc.tile_pool(name="big", bufs=1) as big,
            tc.tile_pool(name="sb", bufs=sb_bufs) as sb,
            tc.tile_pool(name="pt", bufs=pt_bufs) as pt_pool,
            tc.tile_pool(name="small", bufs=small_bufs) as small,
            tc.tile_pool(name="ps1", bufs=1, space="PSUM") as ps1,
            tc.tile_pool(name="ps2", bufs=2, space="PSUM") as ps2,
            tc.tile_pool(name="dram", bufs=2, space="DRAM") as dram,
        ):
            ps = ps1 if v2 else ps2  # v1 keeps its bufs=2 pool as "ps"
            # load constants once
            xs = const.tile([128, 8, T], bf16, tag="xs")
            for cc in range(8):
                nc.sync.dma_start(
                    out=xs[:, cc, :],
                    in_=xT_e.rearrange("(c p) t -> p c t", p=128)[:, cc, :])
            wq_s = const.tile([128, 8, CL], bf16, tag="wq")
            wk_s = const.tile([128, 8, CL], bf16, tag="wk")
            wv_s = const.tile([128, 8, CL], bf16, tag="wv")
            for w_s, w_e in ((wq_s, wq_e), (wk_s, wk_e), (wv_s, wv_e)):
                for cc in range(8):
                    nc.sync.dma_start(
                        out=w_s[:, cc, :],
                        in_=w_e.rearrange("(c p) n -> p c n", p=128)[:, cc, :])
            wp_s = const.tile([128, 4, C], bf16, tag="wp")
            for cc in range(4):
                nc.sync.dma_start(
                    out=wp_s[:, cc, :],
                    in_=wp_e.rearrange("(c p) n -> p c n", p=128)[:, cc, :])
            bqs = const.tile([128, 4], f32, tag="bq")
            nc.sync.dma_start(out=bqs[:], in_=bq_e)
            bks = const.tile([128, 4], f32, tag="bk")
            nc.sync.dma_start(out=bks[:], in_=bk_e)
            bvs = const.tile([128, CL], f32, tag="bv")
            nc.sync.dma_start(out=bvs[:], in_=bv_e.to_broadcast([128, CL]))
            pbs = const.tile([128, C], f32, tag="pb")
            nc.sync.dma_start(out=pbs[:], in_=pb_e.to_broadcast([128, C]))
            tris = const.tile([128, 2048], bf16, tag="tri")
            nc.sync.dma_start(out=tris[:], in_=tri_e)
            vts = []
            for n in range(4):
                vts.append(big.tile([128, HL, 4, 128], bf16, tag=f"v{n}",
                                    name=f"v{n}"))
                nc.vector.memset(vts[n][:, :, :, DH:128], 1.0)

            ext = dict(xs=xs, wq_s=wq_s, wk_s=wk_s, wv_s=wv_s, wp_s=wp_s,
                       bqs=bqs, bks=bks, bvs=bvs, pbs=pbs, tris=tris,
                       vts=vts, out=out_e, snk=snk_e,
                       proj_bf16=proj_bf16,
                       single_core=single_core, no_rs=no_rs, skip=skip,
                       norm_dma=norm_dma, wide_exp=wide_exp,
                       defer_k=defer_k, qk1024=qk1024,
                       narrowmask=narrowmask, ldwshare=ldwshare)
            pools = dict(sb=sb, big=big, ps=ps, ps1=ps1, ps2=ps2, pt=pt_pool,
                         small=small, dram=dram)
            body = build_body_v2 if v2 else build_body
            if loop_n:
                hints = (mybir.EngineType.PE, mybir.EngineType.DVE,
                         mybir.EngineType.Activation, mybir.EngineType.SP,
                         mybir.EngineType.Pool)
                with tc.For_i(0, loop_n, 1, hint_engines=hints,
                              staggered_reset=stagger):
                    for _r in range(body_reps):
                        body(nc, tc, ext, pools)
            else:
                for r in range(reps):
                    body(nc, tc, ext, pools)

    nc.compile()
    return nc


def prep_shards(x, qkv_w, qkv_b, proj_w, proj_b):
    """Host-side sharding + layout prep. Returns in_maps for 8 cores."""
    kr = np.arange(128)[:, None]
    qr = np.arange(512)[None, :]
    tri1 = (qr >= kr).astype(np.float32)          # canonical triangle [128,512]
    pad = np.ones((128, 1), np.float32)
    # packed per-diagonal-pair masks matching the contiguous S layout:
    # dp0 widths (512, 384), dp1 widths (256, 128); rest padded with 1.0
    trip0 = np.concatenate(
        [tri1, tri1[:, 0:384], np.repeat(pad, 128, 1)], axis=1)
    trip1 = np.concatenate(
        [tri1[:, 0:256], tri1[:, 0:128], np.repeat(pad, 640, 1)], axis=1)
    tri = np.concatenate([trip0, trip1], axis=1).astype(BF)
    x = np.asarray(x, np.float32)
    qkv_w = np.asarray(qkv_w, np.float32)
    qkv_b = np.asarray(qkv_b, np.float32)
    proj_w = np.asarray(proj_w, np.float32)
    proj_b = np.asarray(proj_b, np.float32)

    in_maps = []
    for core in range(8):
        b, g = core // 2, core % 2
        hsl = slice(g * CL, (g + 1) * CL)
        wq = qkv_w[0 * C:1 * C][hsl]
        wk = qkv_w[1 * C:2 * C][hsl]
        wv = qkv_w[2 * C:3 * C][hsl]
        in_maps.append({
            "xT": np.ascontiguousarray(x[b].T).astype(BF),
            "wq": np.ascontiguousarray(wq.T).astype(BF),
            "wk": np.ascontiguousarray(wk.T).astype(BF),
            "wv": np.ascontiguousarray(wv.T).astype(BF),
            "wp": np.ascontiguousarray(proj_w[:, hsl].T).astype(BF),
            "bq": np.ascontiguousarray(
                qkv_b[0 * C:1 * C][hsl].reshape(4, 128).T).astype(np.float32),
            "bk": np.ascontiguousarray(
                qkv_b[1 * C:2 * C][hsl].reshape(4, 128).T).astype(np.float32),
            "bv": qkv_b[2 * C:3 * C][hsl].reshape(1, CL).astype(np.float32),
            "pb": (proj_b if g == 0 else np.zeros_like(proj_b)
                   ).reshape(1, C).astype(np.float32),
            "tri": tri,
        })
    return in_maps


def assemble(results):
    # chunked ReduceScatter: per q-tile chunk of 512 rows, rank 0 holds the
    # first 256 reduced rows, rank 1 the last 256
    out = np.empty((B, T, C), np.float32)
    for b in range(B):
        lo = results[2 * b]["out"]
        hi = results[2 * b + 1]["out"]
        for qt in range(4):
            out[b, qt * 512:qt * 512 + 256] = lo[qt * 256:(qt + 1) * 256]
            out[b, qt * 512 + 256:(qt + 1) * 512] = hi[qt * 256:(qt + 1) * 256]
    return out


_CACHE = {}


def _numpy_fallback(x, qkv_w, qkv_b, proj_w, proj_b, mask):
    x = np.asarray(x, np.float32)
    qkv = x @ np.asarray(qkv_w, np.float32).T + np.asarray(qkv_b, np.float32)
    qkv = qkv.reshape(B, T, 3, H, DH).transpose(2, 0, 3, 1, 4)
    q, k, v = qkv[0], qkv[1], qkv[2]
    att = np.einsum("bhqd,bhkd->bhqk", q, k) * (DH ** -0.5)
    att = np.where(np.asarray(mask), att, -np.inf)
    att = att - att.max(axis=-1, keepdims=True)
    att = np.exp(att)
    att /= att.sum(axis=-1, keepdims=True)
    o = np.einsum("bhqk,bhkd->bhqd", att, v)
    o = o.transpose(0, 2, 1, 3).reshape(B, T, C)
    return (o @ np.asarray(proj_w, np.float32).T
            + np.asarray(proj_b, np.float32)).astype(np.float32)


def kernel(x, qkv_w, qkv_b, proj_w, proj_b, mask):
    causal = np.tril(np.ones((T, T), dtype=bool))
    if not np.array_equal(np.asarray(mask).reshape(T, T), causal):
        return _numpy_fallback(x, qkv_w, qkv_b, proj_w, proj_b, mask)

    if "nc" not in _CACHE:
        _CACHE["nc"] = build_graph(reps=1)
    nc = _CACHE["nc"]
    in_maps = prep_shards(x, qkv_w, qkv_b, proj_w, proj_b)
    res = run_bass_kernel_spmd(nc, in_maps, core_ids=list(range(8)))
    return assemble(res.results)



# revision 24
# speedup vs baseline: 1.0593x; 1.0593x over previous
"""Multi-head causal self-attention on 8 Trainium2 NeuronCores.

Problem: B=4, T=2048, C=1024, H=16 heads (DH=64), causal mask, fp32 I/O.

Sharding: core i handles batch b=i//2 and head-group g=i%2 (8 heads).
Per-core compute (bf16 matmuls, fp32 accumulation):
  - QKV projection for its 8 heads:  qT/kT in [d', t] layout, V in [t, d']
    layout with an appended ones-column (gives softmax row-sums for free
    during the AV matmul).
  - Causal attention: S^T = kT^T @ qT per (128-key, 512-query) block,
    exp on ScalarE straight out of PSUM (batched over 2 banks), triangular
    masks applied multiplicatively on VectorE for diagonal blocks, then
    O^T (+row-sums) accumulated in PSUM via the AV matmul.
  - Normalization by reciprocal row-sums (broadcast via a DRAM bounce).
  - Output projection partial product, bias on g=0 cores only, then a
    pair-wise ReduceScatter sums the two head-groups of each batch and
    leaves each core with half the rows of its batch's output.
Host assembles the full [4, 2048, 1024] output from the 8 shards.
"""
import sys

if "/opt/trn_rl_repo" not in sys.path:
    sys.path.insert(0, "/opt/trn_rl_repo")

import numpy as np
import ml_dtypes

import concourse.mybir as mybir
import concourse.tile as tile
from concourse import bacc
from concourse.bass_utils import run_bass_kernel_spmd

B, T, C, H = 4, 2048, 1024, 16
DH = C // H              # 64
HL = H // 2              # 8 heads per core
CL = HL * DH             # 512 local channels
THALF = T // 2           # 1024 rows of output per core after ReduceScatter

bf16 = mybir.dt.bfloat16
f32 = mybir.dt.float32
BF = ml_dtypes.bfloat16

REPLICA_GROUPS = [[0, 1], [2, 3], [4, 5], [6, 7]]


def build_body(nc, tc, ext, pools, rep_tag=""):
    """Emit one full forward pass. `ext` holds external APs, `pools` the
    shared tile pools (so repeated bodies reuse SBUF/PSUM slots)."""
    sb, big, ps, pt_pool, small, dram = (
        pools["sb"], pools["big"], pools["ps"], pools["pt"], pools["small"],
        pools["dram"],
    )
    Exp = mybir.ActivationFunctionType.Exp
    mul = mybir.AluOpType.mult

    xs = ext["xs"]; wq_s = ext["wq_s"]; wk_s = ext["wk_s"]; wv_s = ext["wv_s"]
    wp_s = ext["wp_s"]; bqs = ext["bqs"]; bks = ext["bks"]; bvs = ext["bvs"]
    pbs = ext["pbs"]; tris = ext["tris"]
    out_ext = ext["out"]

    # ---- working tiles for this pass ----
    # per-t-chunk tiles so attention(qt) only depends on the chunks it reads
    vts = ext["vts"]
    qts, kts = [], []
    for n in range(4):
        qt_n = big.tile([128, 4, 512], bf16, tag=f"qT{n}")
        kt_n = big.tile([128, 4, 512], bf16, tag=f"kT{n}")
        qts.append(qt_n)
        kts.append(kt_n)
    pdt = bf16 if ext.get("proj_bf16") else f32
    proj_dram = dram.tile([T, C], pdt, tag="proj")
    rs_out = dram.tile([THALF, C], pdt, tag="rs")

    def emit_qkv_chunk(n):
        # qT/kT: out[d' 128, t 512] = sum_cc w[:, cc, d'-slab].T @ xT[:, cc, t]
        for m in range(4):              # d' slabs of 128
            for w_s, dst, bias in ((wq_s, qts[n], bqs), (wk_s, kts[n], bks)):
                ps_t = ps.tile([128, 512], f32, tag="P")
                for cc in range(8):
                    nc.tensor.matmul(
                        ps_t[:],
                        lhsT=w_s[:, cc, m * 128:(m + 1) * 128],
                        rhs=xs[:, cc, n * 512:(n + 1) * 512],
                        start=(cc == 0), stop=(cc == 7),
                    )
                nc.vector.tensor_tensor(
                    dst[:, m, :], ps_t[:],
                    bias[:, m:m + 1].to_broadcast([128, 512]),
                    mybir.AluOpType.add)
        # V: out[t 128, d' 512] = sum_cc xT[:, cc, tt].T @ wv[:, cc, :]
        for tl in range(4):
            tt = 4 * n + tl
            ps_t = ps.tile([128, 512], f32, tag="P")
            for cc in range(8):
                nc.tensor.matmul(
                    ps_t[:],
                    lhsT=xs[:, cc, tt * 128:(tt + 1) * 128],
                    rhs=wv_s[:, cc, :],
                    start=(cc == 0), stop=(cc == 7),
                )
            nc.vector.tensor_tensor(
                vts[n][:, :, tl, 0:DH],
                ps_t[:].rearrange("p (h d) -> p h d", h=HL),
                bvs[:].rearrange("p (h d) -> p h d", h=HL),
                mybir.AluOpType.add,
            )

    if "attn" in ext.get("skip", ()):
        # sink qkv outputs so DCE keeps the QKV phase
        snk = ext["snk"]
        for n in range(4):
            emit_qkv_chunk(n)
            nc.sync.dma_start(out=snk[0:128, n * 2048:(n + 1) * 2048],
                              in_=qts[n][:].rearrange("p a b -> p (a b)"))
        return

    def emit_proj(qt, attnT):
        # output projection for this q-tile
        for tt in range(4):
            t0 = qt * 512 + tt * 128
            for cn in range(2):
                ps_P = ps.tile([128, 512], f32, tag="P", name="P")
                for j in range(4):
                    nc.tensor.matmul(
                        ps_P[:],
                        lhsT=attnT[:, j, tt * 128:(tt + 1) * 128],
                        rhs=wp_s[:, j, cn * 512:(cn + 1) * 512],
                        start=(j == 0), stop=(j == 3),
                    )
                ot = small.tile([128, 512], pdt, tag="ot", name="ot")
                nc.vector.tensor_tensor(
                    ot[:], ps_P[:], pbs[:, cn * 512:(cn + 1) * 512],
                    mybir.AluOpType.add)
                nc.sync.dma_start(
                    out=proj_dram[t0:t0 + 128, cn * 512:(cn + 1) * 512],
                    in_=ot[:])

        # pairwise ReduceScatter + output DMA for this q-tile's rows
        if ext.get("no_rs"):
            nc.sync.dma_start(
                out=rs_out[qt * 256:(qt + 1) * 256, :],
                in_=proj_dram[qt * 512:qt * 512 + 256, :])
        elif ext.get("single_core"):
            nc.gpsimd.dma_start(
                out=out_ext[qt * 256:(qt + 1) * 256, :],
                in_=proj_dram[qt * 512:qt * 512 + 256, :])
        else:
            nc.gpsimd.collective_compute(
                "ReduceScatter",
                mybir.AluOpType.add,
                replica_groups=REPLICA_GROUPS,
                ins=[proj_dram[qt * 512:(qt + 1) * 512, :].opt()],
                outs=[rs_out[qt * 256:(qt + 1) * 256, :].opt()],
            )
            if ext.get("proj_bf16"):
                for hh in range(2):
                    r0 = qt * 256 + hh * 128
                    stg = small.tile([128, C], bf16, tag="stg", name="stg")
                    nc.sync.dma_start(out=stg[:], in_=rs_out[r0:r0 + 128, :])
                    stf = small.tile([128, C], f32, tag="stf", name="stf")
                    nc.vector.tensor_copy(stf[:], stg[:])
                    nc.sync.dma_start(out=out_ext[r0:r0 + 128, :], in_=stf[:])
            else:
                nc.sync.dma_start(
                    out=out_ext[qt * 256:(qt + 1) * 256, :],
                    in_=rs_out[qt * 256:(qt + 1) * 256, :])

    dk = ext.get("defer_k", 1)
    # ---- QKV production, then attention ----
    if ext.get("ldwshare"):
        # Q/K reordered so consecutive MMs share an identical lhsT AP:
        # for each (weight, slab, cc) load, stream two n-chunks into the
        # two P-tag PSUM slots. If the toolchain elides repeated weight
        # loads, half the Q/K LDWs disappear.
        for np2 in range(2):                    # n-chunk pairs (0,1), (2,3)
            n0, n1 = 2 * np2, 2 * np2 + 1
            for m in range(4):
                for w_s, dsts, bias in ((wq_s, qts, bqs), (wk_s, kts, bks)):
                    ps_a = ps.tile([128, 512], f32, tag="P", name="Pa")
                    ps_b = ps.tile([128, 512], f32, tag="P", name="Pb")
                    for cc in range(8):
                        lhs = w_s[:, cc, m * 128:(m + 1) * 128]
                        nc.tensor.matmul(
                            ps_a[:], lhsT=lhs,
                            rhs=xs[:, cc, n0 * 512:(n0 + 1) * 512],
                            start=(cc == 0), stop=(cc == 7))
                        nc.tensor.matmul(
                            ps_b[:], lhsT=lhs,
                            rhs=xs[:, cc, n1 * 512:(n1 + 1) * 512],
                            start=(cc == 0), stop=(cc == 7))
                    for pst, n in ((ps_a, n0), (ps_b, n1)):
                        nc.vector.tensor_tensor(
                            dsts[n][:, m, :], pst[:],
                            bias[:, m:m + 1].to_broadcast([128, 512]),
                            mybir.AluOpType.add)
        for n in range(4):                      # V chains unchanged
            for tl in range(4):
                tt = 4 * n + tl
                ps_t = ps.tile([128, 512], f32, tag="P", name="Pv2")
                for cc in range(8):
                    nc.tensor.matmul(
                        ps_t[:],
                        lhsT=xs[:, cc, tt * 128:(tt + 1) * 128],
                        rhs=wv_s[:, cc, :],
                        start=(cc == 0), stop=(cc == 7),
                    )
                nc.vector.tensor_tensor(
                    vts[n][:, :, tl, 0:DH],
                    ps_t[:].rearrange("p (h d) -> p h d", h=HL),
                    bvs[:].rearrange("p (h d) -> p h d", h=HL),
                    mybir.AluOpType.add,
                )
    elif ext.get("qk1024"):
        # Q/K with N=1024 moving operand: halves the Q/K matmul count.
        # Chains use the (idle during QKV) "S" tag's [128,1024] PSUM slots.
        for m in range(4):
            for w_s, dsts, bias in ((wq_s, qts, bqs), (wk_s, kts, bks)):
                for np2 in range(2):            # n-chunk pairs (0,1), (2,3)
                    ps_t = ps.tile([128, 1024], f32, tag="S", name="Pqk")
                    for cc in range(8):
                        nc.tensor.matmul(
                            ps_t[:],
                            lhsT=w_s[:, cc, m * 128:(m + 1) * 128],
                            rhs=xs[:, cc, np2 * 1024:(np2 + 1) * 1024],
                            start=(cc == 0), stop=(cc == 7),
                        )
                    for e in range(2):
                        n = 2 * np2 + e
                        nc.vector.tensor_tensor(
                            dsts[n][:, m, :], ps_t[:, e * 512:(e + 1) * 512],
                            bias[:, m:m + 1].to_broadcast([128, 512]),
                            mybir.AluOpType.add)
        for n in range(4):                      # V chains unchanged
            for tl in range(4):
                tt = 4 * n + tl
                ps_t = ps.tile([128, 512], f32, tag="P", name="Pv")
                for cc in range(8):
                    nc.tensor.matmul(
                        ps_t[:],
                        lhsT=xs[:, cc, tt * 128:(tt + 1) * 128],
                        rhs=wv_s[:, cc, :],
                        start=(cc == 0), stop=(cc == 7),
                    )
                nc.vector.tensor_tensor(
                    vts[n][:, :, tl, 0:DH],
                    ps_t[:].rearrange("p (h d) -> p h d", h=HL),
                    bvs[:].rearrange("p (h d) -> p h d", h=HL),
                    mybir.AluOpType.add,
                )
    else:
        for n in range(4):
            emit_qkv_chunk(n)
    attnTs = {}
    for qt in range(4):
        attnT = sb.tile([128, 4, 512], bf16, tag="attnT", name="attnT")
        kmax = 4 * (qt + 1)
        for h in range(HL):
            j, half = h // 2, (h % 2) * 64
            ps_O = ps.tile([128, 512], f32, tag="O")
            for p in range(kmax // 2):
                kc0 = 2 * p
                # widths: diagonal chunks only need the causally-valid
                # query suffix (d = kc - 4*qt -> width 512 - 128*d)
                ws = []
                for e in range(2):
                    d = (kc0 + e) - 4 * qt
                    ws.append(512 if d < 0 else 512 - 128 * d)
                # pack the two S blocks contiguously: e=0 at [0:w0],
                # e=1 at [w0:w0+w1] (no PSUM gap for the exp to read)
                offs = [0, ws[0]]
                ps_S = ps.tile([128, 1024], f32, tag="S")
                for e in range(2):
                    kc, w = kc0 + e, ws[e]
                    nc.tensor.matmul(
                        ps_S[:, offs[e]:offs[e] + w],
                        lhsT=kts[kc // 4][half:half + 64, j,
                                          (kc % 4) * 128:(kc % 4 + 1) * 128],
                        rhs=qts[qt][half:half + 64, j, 512 - w:512],
                        start=True, stop=True,
                    )
                pt = pt_pool.tile([128, 1024], bf16, tag="pt")
                espan = ws[0] + ws[1]
                nc.scalar.activation(pt[:, 0:espan], ps_S[:, 0:espan], Exp,
                                     scale=DH ** -0.5)
                if ext.get("narrowmask"):
                    # only the first 128 cols of a diagonal chunk straddle
                    # the causal boundary; mask just those
                    for e in range(2):
                        if (kc0 + e) >= 4 * qt:
                            nc.vector.tensor_tensor(
                                pt[:, offs[e]:offs[e] + 128],
                                pt[:, offs[e]:offs[e] + 128],
                                tris[:, 0:128], mul)
                elif kc0 >= 4 * qt:      # diagonal pair -> causal mask
                    dp = (kc0 - 4 * qt) // 2
                    nc.vector.tensor_tensor(
                        pt[:, 0:espan], pt[:, 0:espan],
                        tris[:, dp * 1024:dp * 1024 + espan], mul)
                for e in range(2):
                    kc, w = kc0 + e, ws[e]
                    nc.tensor.matmul(
                        ps_O[:, 512 - w:512],
                        lhsT=vts[kc // 4][:, h, kc % 4, :],
                        rhs=pt[:, offs[e]:offs[e] + w],
                        start=(kc == 0), stop=(kc == kmax - 1),
                    )
            rb = small.tile([64, 512], bf16, tag="rb")
            with nc.allow_low_precision(reason="bf16 recip, bf16 mult"):
                nc.vector.reciprocal(rb[:], ps_O[64:128, :])
            nc.vector.tensor_tensor(attnT[half:half + 64, j, :],
                                    ps_O[0:64, :], rb[:], mul)
        attnTs[qt] = attnT

        if "proj" in ext.get("skip", ()):
            snk = ext["snk"]
            nc.sync.dma_start(out=snk[qt * 128:(qt + 1) * 128, 0:2048],
                              in_=attnT[:].rearrange("p a b -> p (a b)"))
            continue
        if dk:
            if qt >= dk:
                emit_proj(qt - dk, attnTs[qt - dk])
            continue
        emit_proj(qt, attnT)



    if dk and "proj" not in ext.get("skip", ()):
        for r in range(4 - dk, 4):
            emit_proj(r, attnTs[r])


def build_body_v2(nc, tc, ext, pools):
    """Software-pipelined body: head-pair attention units with concurrent
    row-group S matmuls, pair-packed PSUM score tiles (one exp per kc-pair),
    and QKV/proj chains interleaved between attention units as PE filler."""
    sb, big, ps1, ps2, pt_pool, small, dram = (
        pools["sb"], pools["big"], pools["ps1"], pools["ps2"], pools["pt"],
        pools["small"], pools["dram"],
    )
    Exp = mybir.ActivationFunctionType.Exp
    mul = mybir.AluOpType.mult
    add = mybir.AluOpType.add
    scale = DH ** -0.5

    xs = ext["xs"]; wq_s = ext["wq_s"]; wk_s = ext["wk_s"]; wv_s = ext["wv_s"]
    wp_s = ext["wp_s"]; bqs = ext["bqs"]; bks = ext["bks"]; bvs = ext["bvs"]
    pbs = ext["pbs"]; tris = ext["tris"]
    out_ext = ext["out"]
    vts = ext["vts"]

    qts, kts = [], []
    for n in range(4):
        qts.append(big.tile([128, 4, 512], bf16, tag=f"qT{n}", name=f"qT{n}"))
        kts.append(big.tile([128, 4, 512], bf16, tag=f"kT{n}", name=f"kT{n}"))
    pdt = bf16 if ext.get("proj_bf16") else f32
    proj_dram = dram.tile([T, C], pdt, tag="proj", name="proj")
    rs_out = dram.tile([THALF, C], pdt, tag="rs", name="rs")

    def qkv_chunk_units(n):
        units = []
        for m in range(4):
            for w_s, dsts, bias in ((wq_s, qts, bqs), (wk_s, kts, bks)):
                def u(m=m, w_s=w_s, dsts=dsts, bias=bias, n=n):
                    ps_t = ps2.tile([128, 512], f32, tag="P", name="Pqk")
                    for cc in range(8):
                        nc.tensor.matmul(
                            ps_t[:],
                            lhsT=w_s[:, cc, m * 128:(m + 1) * 128],
                            rhs=xs[:, cc, n * 512:(n + 1) * 512],
                            start=(cc == 0), stop=(cc == 7),
                        )
                    nc.vector.tensor_tensor(
                        dsts[n][:, m, :], ps_t[:],
                        bias[:, m:m + 1].to_broadcast([128, 512]), add)
                units.append(u)
        for tl in range(4):
            def u(tl=tl, n=n):
                tt = 4 * n + tl
                ps_t = ps2.tile([128, 512], f32, tag="P", name="Pv")
                for cc in range(8):
                    nc.tensor.matmul(
                        ps_t[:],
                        lhsT=xs[:, cc, tt * 128:(tt + 1) * 128],
                        rhs=wv_s[:, cc, :],
                        start=(cc == 0), stop=(cc == 7),
                    )
                nc.vector.tensor_tensor(
                    vts[n][:, :, tl, 0:DH],
                    ps_t[:].rearrange("p (h d) -> p h d", h=HL),
                    bvs[:].rearrange("p (h d) -> p h d", h=HL),
                    add,
                )
            units.append(u)
        return units

    def proj_units(qt, attnT):
        units = []
        for tt in range(4):
            t0 = qt * 512 + tt * 128
            for cn in range(2):
                def u(tt=tt, cn=cn, t0=t0, attnT=attnT):
                    ps_P = ps2.tile([128, 512], f32, tag="P", name="Pp")
                    for jj in range(4):
                        nc.tensor.matmul(
                            ps_P[:],
                            lhsT=attnT[:, jj, tt * 128:(tt + 1) * 128],
                            rhs=wp_s[:, jj, cn * 512:(cn + 1) * 512],
                            start=(jj == 0), stop=(jj == 3),
                        )
                    ot = small.tile([128, 512], pdt, tag="ot", name="ot")
                    nc.vector.tensor_tensor(
                        ot[:], ps_P[:], pbs[:, cn * 512:(cn + 1) * 512], add)
                    nc.sync.dma_start(
                        out=proj_dram[t0:t0 + 128, cn * 512:(cn + 1) * 512],
                        in_=ot[:])
                units.append(u)
        return units

    def rs_unit(qt):
        def u(qt=qt):
            if ext.get("no_rs"):
                nc.sync.dma_start(
                    out=rs_out[qt * 256:(qt + 1) * 256, :],
                    in_=proj_dram[qt * 512:qt * 512 + 256, :])
            else:
                nc.gpsimd.collective_compute(
                    "ReduceScatter",
                    mybir.AluOpType.add,
                    replica_groups=REPLICA_GROUPS,
                    ins=[proj_dram[qt * 512:(qt + 1) * 512, :].opt()],
                    outs=[rs_out[qt * 256:(qt + 1) * 256, :].opt()],
                )
                if ext.get("proj_bf16"):
                    for hh in range(2):
                        r0 = qt * 256 + hh * 128
                        stg = small.tile([128, C], bf16, tag="stg",
                                         name="stg")
                        nc.sync.dma_start(out=stg[:], in_=rs_out[r0:r0 + 128, :])
                        stf = small.tile([128, C], f32, tag="stf", name="stf")
                        nc.vector.tensor_copy(stf[:], stg[:])
                        nc.sync.dma_start(out=out_ext[r0:r0 + 128, :],
                                          in_=stf[:])
                else:
                    nc.sync.dma_start(
                        out=out_ext[qt * 256:(qt + 1) * 256, :],
                        in_=rs_out[qt * 256:(qt + 1) * 256, :])
        return [u]

    def attn_unit(qt, j, attnT):
        # Per-kc processing with ping-pong score tiles: halves side-by-side
        # in one [128,1024] tile (half0 at [0:w], half1 at [512:512+w]), one
        # exp per kc covering both halves, so exp(kc) overlaps S MMs(kc+1).
        # The two S MMs target row-groups (0,·)/(64,·) and run concurrently.
        kmax = 4 * (qt + 1)
        ps_O0 = ps1.tile([128, 512], f32, tag="O0", name="O0")
        ps_O1 = ps1.tile([128, 512], f32, tag="O1", name="O1")

        def width(kc):
            d = kc - 4 * qt
            return 512 if d < 0 else 512 - 128 * d

        def emit_S(kc):
            w = width(kc)
            ps_S = ps1.tile([128, 1024], f32, tag=f"S{kc % 2}",
                            name="ps_S")
            for half, base in ((0, 0), (64, 512)):
                nc.tensor.matmul(
                    ps_S[:, base:base + w],
                    lhsT=kts[kc // 4][half:half + 64, j,
                                      (kc % 4) * 128:(kc % 4 + 1) * 128],
                    rhs=qts[qt][half:half + 64, j, 512 - w:512],
                    start=True, stop=True,
                )
            return ps_S

        # s_ahead=2: PE stream per link is [S(kc+2), AV(kc-2)] — the AV lags
        # the exp that produced its pt by two full links, so the PE never
        # parks on a just-issued exp/mask completion, and exp(kc+1)'s S is
        # already in PSUM when exp(kc) retires -> ACT streams back-to-back.
        # s_ahead=1: [S(kc+2), AV(kc)] (PE waits each fresh exp).
        # s_ahead=0: [S(kc), AV(kc)] after exp(kc) (original chain).
        s_ahead = ext.get("s_ahead", 0)
        no_av = "av" in ext.get("skip", ())
        no_exp = "exp" in ext.get("skip", ())
        av_lag = 2 if s_ahead == 2 else 0

        av64 = ext.get("av64", 1)

        def emit_AV(kc, pt):
            w = width(kc)
            n, lc = kc // 4, kc % 4
            if not av64:
                for hi, (base, ps_O) in enumerate(((0, ps_O0), (512, ps_O1))):
                    nc.tensor.matmul(
                        ps_O[:, 512 - w:512],
                        lhsT=vts[n][:, 2 * j + hi, lc, :],
                        rhs=pt[:, base:base + w],
                        start=(kc == 0), stop=(kc == kmax - 1),
                    )
                return
            # Key-split AV: 4 K=64 matmuls with row-groups alternating
            # r0/r64 so every LDWEIGHTS pulls ahead of the other-half MM
            # in flight, and disjoint-row disjoint-bank pairs overlap.
            # (hi, key-half, dest): O0 and O1 each accumulate both halves.
            seq = ((0, 0, ps_O0), (1, 64, ps_O1), (1, 0, ps_O1),
                   (0, 64, ps_O0))
            for i, (hi, rb, ps_O) in enumerate(seq):
                nc.tensor.matmul(
                    ps_O[:, 512 - w:512],
                    lhsT=vts[n][rb:rb + 64, 2 * j + hi, lc, :],
                    rhs=pt[rb:rb + 64, hi * 512:hi * 512 + w],
                    start=(kc == 0 and i < 2), stop=(kc == kmax - 1 and i >= 2),
                )

        def emit_exp_mask(kc, ps_S):
            w = width(kc)
            pt = pt_pool.tile([128, 1024], bf16, tag="pt", name="pt")
            nc.scalar.activation(pt[:, 0:512 + w], ps_S[:, 0:512 + w], Exp,
                                 scale=scale)
            if kc >= 4 * qt:
                for base in (0, 512):
                    nc.vector.tensor_tensor(
                        pt[:, base:base + 128], pt[:, base:base + 128],
                        tris[:, 0:128], mul)
            return pt

        if s_ahead == 3 and not (no_av or no_exp):
            # Bunched pipeline: per 2-link iteration emit
            #   exp(kc0) exp(kc1) | S(kc0+2) S(kc0+3) | AV(kc0-2) AV(kc0-1)
            # Same-type matmuls stay adjacent so LDWEIGHTS pulls ahead via
            # the background buffer; only two S<->AV boundaries per 2 links
            # pay an exposed weight load. AV lags its exp by a full
            # iteration so the PE never parks on a fresh exp/mask sem.
            ps_Ss = {0: emit_S(0), 1: emit_S(1)}
            pts = {}
            for kc0 in range(0, kmax, 2):
                for kc in (kc0, kc0 + 1):
                    pts[kc] = emit_exp_mask(kc, ps_Ss.pop(kc))
                for kc in (kc0 + 2, kc0 + 3):
                    if kc < kmax:
                        ps_Ss[kc] = emit_S(kc)
                for kc in (kc0 - 2, kc0 - 1):
                    if kc >= 0:
                        emit_AV(kc, pts.pop(kc))
            for kc in sorted(pts):
                emit_AV(kc, pts.pop(kc))
            for half, ps_O in ((0, ps_O0), (64, ps_O1)):
                rb = small.tile([64, 512], bf16, tag="rb", name="rb")
                with nc.allow_low_precision(reason="bf16 recip/mult"):
                    nc.vector.reciprocal(rb[:], ps_O[64:128, :])
                nc.vector.tensor_tensor(attnT[half:half + 64, j, :],
                                        ps_O[0:64, :], rb[:], mul)
            return

        ps_Ss = {}
        if s_ahead:
            ps_Ss[0] = emit_S(0)
            if kmax > 1:
                ps_Ss[1] = emit_S(1)
        if no_exp and "pts_static" not in ext:
            # clean PE-side probe: pre-made pt slots, no per-kc writes
            ext["pts_static"] = []
            for i in range(4):
                spt = pt_pool.tile([128, 1024], bf16, tag="pt",
                                   name="spt")
                nc.vector.memset(spt[:], 0.5)
                ext["pts_static"].append(spt)
        pts = {}
        for kc in range(kmax):
            w = width(kc)
            ps_S = ps_Ss.pop(kc) if kc in ps_Ss else emit_S(kc)
            if no_exp:
                pts[kc] = ext["pts_static"][kc % 4]
                if kc - av_lag >= 0:
                    emit_AV(kc - av_lag, pts.pop(kc - av_lag))
                continue
            pt = pt_pool.tile([128, 1024], bf16, tag="pt", name="pt")
            pts[kc] = pt
            if not no_exp:
                # single exp over [0:512+w] spans the [w:512] hole when
                # w<512 (cheaper than a second activation's overhead)
                nc.scalar.activation(pt[:, 0:512 + w], ps_S[:, 0:512 + w],
                                     Exp, scale=scale)
                if kc >= 4 * qt and not no_av:
                    # causal boundary: first 128 cols of the block
                    for base in (0, 512):
                        nc.vector.tensor_tensor(
                            pt[:, base:base + 128], pt[:, base:base + 128],
                            tris[:, 0:128], mul)
            if s_ahead and kc + 2 < kmax:
                ps_Ss[kc + 2] = emit_S(kc + 2)
            if no_av:
                # probe: consume pt cheaply so exps aren't dead code
                pts.pop(kc)
                nc.vector.tensor_tensor(
                    ext["acc"][:], ext["acc"][:], pt[:, 0:8],
                    mybir.AluOpType.max)
                continue
            if no_exp:
                nc.vector.memset(pt[:, 0:8], 1.0)  # allocate pt w/o exp
            if kc - av_lag >= 0:
                emit_AV(kc - av_lag, pts.pop(kc - av_lag))
        if no_av:
            return
        for kc in sorted(pts):
            emit_AV(kc, pts.pop(kc))
        for half, ps_O in ((0, ps_O0), (64, ps_O1)):
            rb = small.tile([64, 512], bf16, tag="rb", name="rb")
            with nc.allow_low_precision(reason="bf16 recip, bf16 mult"):
                nc.vector.reciprocal(rb[:], ps_O[64:128, :])
            nc.vector.tensor_tensor(attnT[half:half + 64, j, :],
                                    ps_O[0:64, :], rb[:], mul)

    # ---- emission: prologue, then attention rounds with filler ----
    from collections import deque
    skip = ext.get("skip", ())
    if "attn" in skip:
        snk = ext["snk"]
        for n in range(4):
            for u in qkv_chunk_units(n):
                u()
            nc.sync.dma_start(out=snk[0:128, n * 2048:(n + 1) * 2048],
                              in_=qts[n][:].rearrange("p a b -> p (a b)"))
        return
    do_qkv = "qkv" not in skip
    do_proj = "proj" not in skip
    no_av = "av" in skip
    if no_av:
        acc = small.tile([128, 8], bf16, tag="acc", name="acc")
        nc.vector.memset(acc[:], 0.0)
        ext["acc"] = acc
    filler = deque()
    if do_qkv:
        c0 = qkv_chunk_units(0)
        if ext.get("ptrim"):
            # prologue: only what attention unit (0,0) needs — Q/K slab m=0
            # and the V chains; the rest fills round 0
            for u in c0[0:2] + c0[8:12]:
                u()
            filler.extend(c0[2:8])
        else:
            for u in c0:
                u()
        filler.extend(qkv_chunk_units(1))
    else:
        # timing-only variant: touch q/k tiles so Tile allocates them
        for n in range(4):
            nc.vector.memset(qts[n][:, :, 0:1], 1.0)
            nc.vector.memset(kts[n][:, :, 0:1], 1.0)
    attnTs = {}
    for qt in range(4):
        attnT = sb.tile([128, 4, 512], bf16, tag="attnT", name="attnT")
        for j in range(4):
            attn_unit(qt, j, attnT)
            npop = (len(filler) + (3 - j)) // (4 - j)
            for _ in range(npop):
                filler.popleft()()
        attnTs[qt] = attnT
        if do_qkv:
            if qt == 0:
                filler.extend(qkv_chunk_units(2))
            elif qt == 1:
                filler.extend(qkv_chunk_units(3))
        if not do_proj:
            snk = ext["snk"]
            if no_av:
                nc.sync.dma_start(out=snk[qt:qt + 1, 0:8],
                                  in_=ext["acc"][0:1, :])
            else:
                nc.sync.dma_start(out=snk[qt * 128:(qt + 1) * 128, 0:2048],
                                  in_=attnT[:].rearrange("p a b -> p (a b)"))
            continue
        if qt == 1:
            filler.extend(proj_units(0, attnTs[0]))
            filler.extend(rs_unit(0))
        elif qt == 2:
            filler.extend(proj_units(1, attnTs[1]))
            filler.extend(rs_unit(1))
            filler.extend(proj_units(2, attnTs[2]))
            filler.extend(rs_unit(2))
    while filler:
        filler.popleft()()
    if do_proj:
        for u in proj_units(3, attnTs[3]) + rs_unit(3):
            u()


def build_graph(reps=1, single_core=False, no_rs=False, skip=(),
                norm_dma=False, wide_exp=False, loop_n=0,
                pt_bufs=4, sb_bufs=2, small_bufs=3, swap_side=False,
                proj_bf16=True, defer_k=1, qk1024=False, narrowmask=False,
                ldwshare=False, stagger=False, body_reps=1, v2=True,
                s_ahead=2, av64=0, ptrim=0):
    nc = bacc.Bacc("TRN2", target_bir_lowering=False, debug=False,
                   num_devices=1 if single_core else 8)
    xT_e = nc.dram_tensor("xT", [C, T], bf16, kind="ExternalInput").ap()
    wq_e = nc.dram_tensor("wq", [C, CL], bf16, kind="ExternalInput").ap()
    wk_e = nc.dram_tensor("wk", [C, CL], bf16, kind="ExternalInput").ap()
    wv_e = nc.dram_tensor("wv", [C, CL], bf16, kind="ExternalInput").ap()
    wp_e = nc.dram_tensor("wp", [CL, C], bf16, kind="ExternalInput").ap()
    bq_e = nc.dram_tensor("bq", [128, 4], f32, kind="ExternalInput").ap()
    bk_e = nc.dram_tensor("bk", [128, 4], f32, kind="ExternalInput").ap()
    bv_e = nc.dram_tensor("bv", [1, CL], f32, kind="ExternalInput").ap()
    pb_e = nc.dram_tensor("pb", [1, C], f32, kind="ExternalInput").ap()
    tri_e = nc.dram_tensor("tri", [128, 2048], bf16, kind="ExternalInput").ap()
    out_e = nc.dram_tensor("out", [THALF, C], f32, kind="ExternalOutput").ap()
    snk_e = (nc.dram_tensor("snk", [512, 8320], bf16, kind="ExternalOutput").ap()
             if skip else None)

    if v2:
        sb_bufs = max(sb_bufs, 4)   # attnT read by proj up to 2 rounds later
        pt_bufs = max(pt_bufs, 4)   # pt must outlive AV lag 2
    with tile.TileContext(nc) as tc:
        if swap_side:
            tc.swap_default_side()
        with (
            tc.tile_pool(name="const", bufs=1) as const,
            tc.tile_pool(name="big", bufs=1) as big,
            tc.tile_pool(name="sb", bufs=sb_bufs) as sb,
            tc.tile_pool(name="pt", bufs=pt_bufs) as pt_pool,
            tc.tile_pool(name="small", bufs=small_bufs) as small,
            tc.tile_pool(name="ps1", bufs=1, space="PSUM") as ps1,
            tc.tile_pool(name="ps2", bufs=2, space="PSUM") as ps2,
            tc.tile_pool(name="dram", bufs=2, space="DRAM") as dram,
        ):
            ps = ps1 if v2 else ps2  # v1 keeps its bufs=2 pool as "ps"
            # load constants once
            xs = const.tile([128, 8, T], bf16, tag="xs")
            for cc in range(8):
                nc.sync.dma_start(
                    out=xs[:, cc, :],
                    in_=xT_e.rearrange("(c p) t -> p c t", p=128)[:, cc, :])
            wq_s = const.tile([128, 8, CL], bf16, tag="wq")
            wk_s = const.tile([128, 8, CL], bf16, tag="wk")
            wv_s = const.tile([128, 8, CL], bf16, tag="wv")
            for w_s, w_e in ((wq_s, wq_e), (wk_s, wk_e), (wv_s, wv_e)):
                for cc in range(8):
                    nc.sync.dma_start(
                        out=w_s[:, cc, :],
                        in_=w_e.rearrange("(c p) n -> p c n", p=128)[:, cc, :])
            wp_s = const.tile([128, 4, C], bf16, tag="wp")
            for cc in range(4):
                nc.sync.dma_start(
                    out=wp_s[:, cc, :],
                    in_=wp_e.rearrange("(c p) n -> p c n", p=128)[:, cc, :])
            bqs = const.tile([128, 4], f32, tag="bq")
            nc.sync.dma_start(out=bqs[:], in_=bq_e)
            bks = const.tile([128, 4], f32, tag="bk")
            nc.sync.dma_start(out=bks[:], in_=bk_e)
            bvs = const.tile([128, CL], f32, tag="bv")
            nc.sync.dma_start(out=bvs[:], in_=bv_e.to_broadcast([128, CL]))
            pbs = const.tile([128, C], f32, tag="pb")
            nc.sync.dma_start(out=pbs[:], in_=pb_e.to_broadcast([128, C]))
            tris = const.tile([128, 2048], bf16, tag="tri")
            nc.sync.dma_start(out=tris[:], in_=tri_e)
            vts = []
            for n in range(4):
                vts.append(big.tile([128, HL, 4, 128], bf16, tag=f"v{n}",
                                    name=f"v{n}"))
                nc.vector.memset(vts[n][:, :, :, DH:128], 1.0)

            ext = dict(xs=xs, wq_s=wq_s, wk_s=wk_s, wv_s=wv_s, wp_s=wp_s,
                       bqs=bqs, bks=bks, bvs=bvs, pbs=pbs, tris=tris,
                       vts=vts, out=out_e, snk=snk_e,
                       proj_bf16=proj_bf16,
                       single_core=single_core, no_rs=no_rs, skip=skip,
                       norm_dma=norm_dma, wide_exp=wide_exp,
                       defer_k=defer_k, qk1024=qk1024,
                       narrowmask=narrowmask, ldwshare=ldwshare,
                       s_ahead=s_ahead, av64=av64, ptrim=ptrim)
            pools = dict(sb=sb, big=big, ps=ps, ps1=ps1, ps2=ps2, pt=pt_pool,
                         small=small, dram=dram)
            body = build_body_v2 if v2 else build_body
            if loop_n:
                hints = (mybir.EngineType.PE, mybir.EngineType.DVE,
                         mybir.EngineType.Activation, mybir.EngineType.SP,
                         mybir.EngineType.Pool)
                with tc.For_i(0, loop_n, 1, hint_engines=hints,
                              staggered_reset=stagger):
                    for _r in range(body_reps):
                        body(nc, tc, ext, pools)
            else:
                for r in range(reps):
                    body(nc, tc, ext, pools)

    nc.compile()
    return nc


def prep_shards(x, qkv_w, qkv_b, proj_w, proj_b):
    """Host-side sharding + layout prep. Returns in_maps for 8 cores."""
    kr = np.arange(128)[:, None]
    qr = np.arange(512)[None, :]
    tri1 = (qr >= kr).astype(np.float32)          # canonical triangle [128,512]
    pad = np.ones((128, 1), np.float32)
    # packed per-diagonal-pair masks matching the contiguous S layout:
    # dp0 widths (512, 384), dp1 widths (256, 128); rest padded with 1.0
    trip0 = np.concatenate(
        [tri1, tri1[:, 0:384], np.repeat(pad, 128, 1)], axis=1)
    trip1 = np.concatenate(
        [tri1[:, 0:256], tri1[:, 0:128], np.repeat(pad, 640, 1)], axis=1)
    tri = np.concatenate([trip0, trip1], axis=1).astype(BF)
    x = np.asarray(x, np.float32)
    qkv_w = np.asarray(qkv_w, np.float32)
    qkv_b = np.asarray(qkv_b, np.float32)
    proj_w = np.asarray(proj_w, np.float32)
    proj_b = np.asarray(proj_b, np.float32)

    in_maps = []
    for core in range(8):
        b, g = core // 2, core % 2
        hsl = slice(g * CL, (g + 1) * CL)
        wq = qkv_w[0 * C:1 * C][hsl]
        wk = qkv_w[1 * C:2 * C][hsl]
        wv = qkv_w[2 * C:3 * C][hsl]
        in_maps.append({
            "xT": np.ascontiguousarray(x[b].T).astype(BF),
            "wq": np.ascontiguousarray(wq.T).astype(BF),
            "wk": np.ascontiguousarray(wk.T).astype(BF),
            "wv": np.ascontiguousarray(wv.T).astype(BF),
            "wp": np.ascontiguousarray(proj_w[:, hsl].T).astype(BF),
            "bq": np.ascontiguousarray(
                qkv_b[0 * C:1 * C][hsl].reshape(4, 128).T).astype(np.float32),
            "bk": np.ascontiguousarray(
                qkv_b[1 * C:2 * C][hsl].reshape(4, 128).T).astype(np.float32),
            "bv": qkv_b[2 * C:3 * C][hsl].reshape(1, CL).astype(np.float32),
            "pb": (proj_b if g == 0 else np.zeros_like(proj_b)
                   ).reshape(1, C).astype(np.float32),
            "tri": tri,
        })
    return in_maps


def assemble(results):
    # chunked ReduceScatter: per q-tile chunk of 512 rows, rank 0 holds the
    # first 256 reduced rows, rank 1 the last 256
    out = np.empty((B, T, C), np.float32)
    for b in range(B):
        lo = results[2 * b]["out"]
        hi = results[2 * b + 1]["out"]
        for qt in range(4):
            out[b, qt * 512:qt * 512 + 256] = lo[qt * 256:(qt + 1) * 256]
            out[b, qt * 512 + 256:(qt + 1) * 512] = hi[qt * 256:(qt + 1) * 256]
    return out


_CACHE = {}


def _numpy_fallback(x, qkv_w, qkv_b, proj_w, proj_b, mask):
    x = np.asarray(x, np.float32)
    qkv = x @ np.asarray(qkv_w, np.float32).T + np.asarray(qkv_b, np.float32)
    qkv = qkv.reshape(B, T, 3, H, DH).transpose(2, 0, 3, 1, 4)
    q, k, v = qkv[0], qkv[1], qkv[2]
    att = np.einsum("bhqd,bhkd->bhqk", q, k) * (DH ** -0.5)
    att = np.where(np.asarray(mask), att, -np.inf)
    att = att - att.max(axis=-1, keepdims=True)
    att = np.exp(att)
    att /= att.sum(axis=-1, keepdims=True)
    o = np.einsum("bhqk,bhkd->bhqd", att, v)
    o = o.transpose(0, 2, 1, 3).reshape(B, T, C)
    return (o @ np.asarray(proj_w, np.float32).T
            + np.asarray(proj_b, np.float32)).astype(np.float32)


def kernel(x, qkv_w, qkv_b, proj_w, proj_b, mask):
    causal = np.tril(np.ones((T, T), dtype=bool))
    if not np.array_equal(np.asarray(mask).reshape(T, T), causal):
        return _numpy_fallback(x, qkv_w, qkv_b, proj_w, proj_b, mask)

    if "nc" not in _CACHE:
        _CACHE["nc"] = build_graph(reps=1)
    nc = _CACHE["nc"]
    in_maps = prep_shards(x, qkv_w, qkv_b, proj_w, proj_b)
    res = run_bass_kernel_spmd(nc, in_maps, core_ids=list(range(8)))
    return assemble(res.results)



# revision 26
# speedup vs baseline: 1.3619x; 1.2857x over previous
"""Multi-head causal self-attention on 8 Trainium2 NeuronCores.

Problem: B=4, T=2048, C=1024, H=16 heads (DH=64), causal mask, fp32 I/O.

Sharding: core i handles batch b=i//2 and head-group g=i%2 (8 heads).
Per-core compute (bf16 matmuls, fp32 accumulation):
  - QKV projection for its 8 heads:  qT/kT in [d', t] layout, V in [t, d']
    layout with an appended ones-column (gives softmax row-sums for free
    during the AV matmul).
  - Causal attention: S^T = kT^T @ qT per (128-key, 512-query) block,
    exp on ScalarE straight out of PSUM (batched over 2 banks), triangular
    masks applied multiplicatively on VectorE for diagonal blocks, then
    O^T (+row-sums) accumulated in PSUM via the AV matmul.
  - Normalization by reciprocal row-sums (broadcast via a DRAM bounce).
  - Output projection partial product, bias on g=0 cores only, then a
    pair-wise ReduceScatter sums the two head-groups of each batch and
    leaves each core with half the rows of its batch's output.
Host assembles the full [4, 2048, 1024] output from the 8 shards.
"""
import sys

if "/opt/trn_rl_repo" not in sys.path:
    sys.path.insert(0, "/opt/trn_rl_repo")

import numpy as np
import ml_dtypes

import concourse.mybir as mybir
import concourse.tile as tile
from concourse import bacc
from concourse.bass_utils import run_bass_kernel_spmd

B, T, C, H = 4, 2048, 1024, 16
DH = C // H              # 64
HL = H // 2              # 8 heads per core
CL = HL * DH             # 512 local channels
THALF = T // 2           # 1024 rows of output per core after ReduceScatter

bf16 = mybir.dt.bfloat16
f32 = mybir.dt.float32
BF = ml_dtypes.bfloat16

REPLICA_GROUPS = [[0, 1], [2, 3], [4, 5], [6, 7]]


def build_body(nc, tc, ext, pools, rep_tag=""):
    """Emit one full forward pass. `ext` holds external APs, `pools` the
    shared tile pools (so repeated bodies reuse SBUF/PSUM slots)."""
    sb, big, ps, pt_pool, small, dram = (
        pools["sb"], pools["big"], pools["ps"], pools["pt"], pools["small"],
        pools["dram"],
    )
    Exp = mybir.ActivationFunctionType.Exp
    mul = mybir.AluOpType.mult

    xs = ext["xs"]; wq_s = ext["wq_s"]; wk_s = ext["wk_s"]; wv_s = ext["wv_s"]
    wp_s = ext["wp_s"]; bqs = ext["bqs"]; bks = ext["bks"]; bvs = ext["bvs"]
    pbs = ext["pbs"]; tris = ext["tris"]
    out_ext = ext["out"]

    # ---- working tiles for this pass ----
    # per-t-chunk tiles so attention(qt) only depends on the chunks it reads
    vts = ext["vts"]
    qts, kts = [], []
    for n in range(4):
        qt_n = big.tile([128, 4, 512], bf16, tag=f"qT{n}")
        kt_n = big.tile([128, 4, 512], bf16, tag=f"kT{n}")
        qts.append(qt_n)
        kts.append(kt_n)
    pdt = bf16 if ext.get("proj_bf16") else f32
    proj_dram = dram.tile([T, C], pdt, tag="proj")
    rs_out = dram.tile([THALF, C], pdt, tag="rs")

    def emit_qkv_chunk(n):
        # qT/kT: out[d' 128, t 512] = sum_cc w[:, cc, d'-slab].T @ xT[:, cc, t]
        for m in range(4):              # d' slabs of 128
            for w_s, dst, bias in ((wq_s, qts[n], bqs), (wk_s, kts[n], bks)):
                ps_t = ps.tile([128, 512], f32, tag="P")
                for cc in range(8):
                    nc.tensor.matmul(
                        ps_t[:],
                        lhsT=w_s[:, cc, m * 128:(m + 1) * 128],
                        rhs=xs[:, cc, n * 512:(n + 1) * 512],
                        start=(cc == 0), stop=(cc == 7),
                    )
                nc.vector.tensor_tensor(
                    dst[:, m, :], ps_t[:],
                    bias[:, m:m + 1].to_broadcast([128, 512]),
                    mybir.AluOpType.add)
        # V: out[t 128, d' 512] = sum_cc xT[:, cc, tt].T @ wv[:, cc, :]
        for tl in range(4):
            tt = 4 * n + tl
            ps_t = ps.tile([128, 512], f32, tag="P")
            for cc in range(8):
                nc.tensor.matmul(
                    ps_t[:],
                    lhsT=xs[:, cc, tt * 128:(tt + 1) * 128],
                    rhs=wv_s[:, cc, :],
                    start=(cc == 0), stop=(cc == 7),
                )
            nc.vector.tensor_tensor(
                vts[n][:, :, tl, 0:DH],
                ps_t[:].rearrange("p (h d) -> p h d", h=HL),
                bvs[:].rearrange("p (h d) -> p h d", h=HL),
                mybir.AluOpType.add,
            )

    if "attn" in ext.get("skip", ()):
        # sink qkv outputs so DCE keeps the QKV phase
        snk = ext["snk"]
        for n in range(4):
            emit_qkv_chunk(n)
            nc.sync.dma_start(out=snk[0:128, n * 2048:(n + 1) * 2048],
                              in_=qts[n][:].rearrange("p a b -> p (a b)"))
        return

    def emit_proj(qt, attnT):
        # output projection for this q-tile
        for tt in range(4):
            t0 = qt * 512 + tt * 128
            for cn in range(2):
                ps_P = ps.tile([128, 512], f32, tag="P", name="P")
                for j in range(4):
                    nc.tensor.matmul(
                        ps_P[:],
                        lhsT=attnT[:, j, tt * 128:(tt + 1) * 128],
                        rhs=wp_s[:, j, cn * 512:(cn + 1) * 512],
                        start=(j == 0), stop=(j == 3),
                    )
                ot = small.tile([128, 512], pdt, tag="ot", name="ot")
                nc.vector.tensor_tensor(
                    ot[:], ps_P[:], pbs[:, cn * 512:(cn + 1) * 512],
                    mybir.AluOpType.add)
                nc.sync.dma_start(
                    out=proj_dram[t0:t0 + 128, cn * 512:(cn + 1) * 512],
                    in_=ot[:])

        # pairwise ReduceScatter + output DMA for this q-tile's rows
        if ext.get("no_rs"):
            nc.sync.dma_start(
                out=rs_out[qt * 256:(qt + 1) * 256, :],
                in_=proj_dram[qt * 512:qt * 512 + 256, :])
        elif ext.get("single_core"):
            nc.gpsimd.dma_start(
                out=out_ext[qt * 256:(qt + 1) * 256, :],
                in_=proj_dram[qt * 512:qt * 512 + 256, :])
        else:
            nc.gpsimd.collective_compute(
                "ReduceScatter",
                mybir.AluOpType.add,
                replica_groups=REPLICA_GROUPS,
                ins=[proj_dram[qt * 512:(qt + 1) * 512, :].opt()],
                outs=[rs_out[qt * 256:(qt + 1) * 256, :].opt()],
            )
            if ext.get("proj_bf16"):
                for hh in range(2):
                    r0 = qt * 256 + hh * 128
                    stg = small.tile([128, C], bf16, tag="stg", name="stg")
                    nc.sync.dma_start(out=stg[:], in_=rs_out[r0:r0 + 128, :])
                    stf = small.tile([128, C], f32, tag="stf", name="stf")
                    nc.vector.tensor_copy(stf[:], stg[:])
                    nc.sync.dma_start(out=out_ext[r0:r0 + 128, :], in_=stf[:])
            else:
                nc.sync.dma_start(
                    out=out_ext[qt * 256:(qt + 1) * 256, :],
                    in_=rs_out[qt * 256:(qt + 1) * 256, :])

    dk = ext.get("defer_k", 1)
    # ---- QKV production, then attention ----
    if ext.get("ldwshare"):
        # Q/K reordered so consecutive MMs share an identical lhsT AP:
        # for each (weight, slab, cc) load, stream two n-chunks into the
        # two P-tag PSUM slots. If the toolchain elides repeated weight
        # loads, half the Q/K LDWs disappear.
        for np2 in range(2):                    # n-chunk pairs (0,1), (2,3)
            n0, n1 = 2 * np2, 2 * np2 + 1
            for m in range(4):
                for w_s, dsts, bias in ((wq_s, qts, bqs), (wk_s, kts, bks)):
                    ps_a = ps.tile([128, 512], f32, tag="P", name="Pa")
                    ps_b = ps.tile([128, 512], f32, tag="P", name="Pb")
                    for cc in range(8):
                        lhs = w_s[:, cc, m * 128:(m + 1) * 128]
                        nc.tensor.matmul(
                            ps_a[:], lhsT=lhs,
                            rhs=xs[:, cc, n0 * 512:(n0 + 1) * 512],
                            start=(cc == 0), stop=(cc == 7))
                        nc.tensor.matmul(
                            ps_b[:], lhsT=lhs,
                            rhs=xs[:, cc, n1 * 512:(n1 + 1) * 512],
                            start=(cc == 0), stop=(cc == 7))
                    for pst, n in ((ps_a, n0), (ps_b, n1)):
                        nc.vector.tensor_tensor(
                            dsts[n][:, m, :], pst[:],
                            bias[:, m:m + 1].to_broadcast([128, 512]),
                            mybir.AluOpType.add)
        for n in range(4):                      # V chains unchanged
            for tl in range(4):
                tt = 4 * n + tl
                ps_t = ps.tile([128, 512], f32, tag="P", name="Pv2")
                for cc in range(8):
                    nc.tensor.matmul(
                        ps_t[:],
                        lhsT=xs[:, cc, tt * 128:(tt + 1) * 128],
                        rhs=wv_s[:, cc, :],
                        start=(cc == 0), stop=(cc == 7),
                    )
                nc.vector.tensor_tensor(
                    vts[n][:, :, tl, 0:DH],
                    ps_t[:].rearrange("p (h d) -> p h d", h=HL),
                    bvs[:].rearrange("p (h d) -> p h d", h=HL),
                    mybir.AluOpType.add,
                )
    elif ext.get("qk1024"):
        # Q/K with N=1024 moving operand: halves the Q/K matmul count.
        # Chains use the (idle during QKV) "S" tag's [128,1024] PSUM slots.
        for m in range(4):
            for w_s, dsts, bias in ((wq_s, qts, bqs), (wk_s, kts, bks)):
                for np2 in range(2):            # n-chunk pairs (0,1), (2,3)
                    ps_t = ps.tile([128, 1024], f32, tag="S", name="Pqk")
                    for cc in range(8):
                        nc.tensor.matmul(
                            ps_t[:],
                            lhsT=w_s[:, cc, m * 128:(m + 1) * 128],
                            rhs=xs[:, cc, np2 * 1024:(np2 + 1) * 1024],
                            start=(cc == 0), stop=(cc == 7),
                        )
                    for e in range(2):
                        n = 2 * np2 + e
                        nc.vector.tensor_tensor(
                            dsts[n][:, m, :], ps_t[:, e * 512:(e + 1) * 512],
                            bias[:, m:m + 1].to_broadcast([128, 512]),
                            mybir.AluOpType.add)
        for n in range(4):                      # V chains unchanged
            for tl in range(4):
                tt = 4 * n + tl
                ps_t = ps.tile([128, 512], f32, tag="P", name="Pv")
                for cc in range(8):
                    nc.tensor.matmul(
                        ps_t[:],
                        lhsT=xs[:, cc, tt * 128:(tt + 1) * 128],
                        rhs=wv_s[:, cc, :],
                        start=(cc == 0), stop=(cc == 7),
                    )
                nc.vector.tensor_tensor(
                    vts[n][:, :, tl, 0:DH],
                    ps_t[:].rearrange("p (h d) -> p h d", h=HL),
                    bvs[:].rearrange("p (h d) -> p h d", h=HL),
                    mybir.AluOpType.add,
                )
    else:
        for n in range(4):
            emit_qkv_chunk(n)
    attnTs = {}
    for qt in range(4):
        attnT = sb.tile([128, 4, 512], bf16, tag="attnT", name="attnT")
        kmax = 4 * (qt + 1)
        for h in range(HL):
            j, half = h // 2, (h % 2) * 64
            ps_O = ps.tile([128, 512], f32, tag="O")
            for p in range(kmax // 2):
                kc0 = 2 * p
                # widths: diagonal chunks only need the causally-valid
                # query suffix (d = kc - 4*qt -> width 512 - 128*d)
                ws = []
                for e in range(2):
                    d = (kc0 + e) - 4 * qt
                    ws.append(512 if d < 0 else 512 - 128 * d)
                # pack the two S blocks contiguously: e=0 at [0:w0],
                # e=1 at [w0:w0+w1] (no PSUM gap for the exp to read)
                offs = [0, ws[0]]
                ps_S = ps.tile([128, 1024], f32, tag="S")
                for e in range(2):
                    kc, w = kc0 + e, ws[e]
                    nc.tensor.matmul(
                        ps_S[:, offs[e]:offs[e] + w],
                        lhsT=kts[kc // 4][half:half + 64, j,
                                          (kc % 4) * 128:(kc % 4 + 1) * 128],
                        rhs=qts[qt][half:half + 64, j, 512 - w:512],
                        start=True, stop=True,
                    )
                pt = pt_pool.tile([128, 1024], bf16, tag="pt")
                espan = ws[0] + ws[1]
                nc.scalar.activation(pt[:, 0:espan], ps_S[:, 0:espan], Exp,
                                     scale=DH ** -0.5)
                if ext.get("narrowmask"):
                    # only the first 128 cols of a diagonal chunk straddle
                    # the causal boundary; mask just those
                    for e in range(2):
                        if (kc0 + e) >= 4 * qt:
                            nc.vector.tensor_tensor(
                                pt[:, offs[e]:offs[e] + 128],
                                pt[:, offs[e]:offs[e] + 128],
                                tris[:, 0:128], mul)
                elif kc0 >= 4 * qt:      # diagonal pair -> causal mask
                    dp = (kc0 - 4 * qt) // 2
                    nc.vector.tensor_tensor(
                        pt[:, 0:espan], pt[:, 0:espan],
                        tris[:, dp * 1024:dp * 1024 + espan], mul)
                for e in range(2):
                    kc, w = kc0 + e, ws[e]
                    nc.tensor.matmul(
                        ps_O[:, 512 - w:512],
                        lhsT=vts[kc // 4][:, h, kc % 4, :],
                        rhs=pt[:, offs[e]:offs[e] + w],
                        start=(kc == 0), stop=(kc == kmax - 1),
                    )
            rb = small.tile([64, 512], bf16, tag="rb")
            with nc.allow_low_precision(reason="bf16 recip, bf16 mult"):
                nc.vector.reciprocal(rb[:], ps_O[64:128, :])
            nc.vector.tensor_tensor(attnT[half:half + 64, j, :],
                                    ps_O[0:64, :], rb[:], mul)
        attnTs[qt] = attnT

        if "proj" in ext.get("skip", ()):
            snk = ext["snk"]
            nc.sync.dma_start(out=snk[qt * 128:(qt + 1) * 128, 0:2048],
                              in_=attnT[:].rearrange("p a b -> p (a b)"))
            continue
        if dk:
            if qt >= dk:
                emit_proj(qt - dk, attnTs[qt - dk])
            continue
        emit_proj(qt, attnT)



    if dk and "proj" not in ext.get("skip", ()):
        for r in range(4 - dk, 4):
            emit_proj(r, attnTs[r])


def build_body_v2(nc, tc, ext, pools):
    """Software-pipelined body: head-pair attention units with concurrent
    row-group S matmuls, pair-packed PSUM score tiles (one exp per kc-pair),
    and QKV/proj chains interleaved between attention units as PE filler."""
    sb, big, ps1, ps2, pt_pool, small, dram = (
        pools["sb"], pools["big"], pools["ps1"], pools["ps2"], pools["pt"],
        pools["small"], pools["dram"],
    )
    Exp = mybir.ActivationFunctionType.Exp
    mul = mybir.AluOpType.mult
    add = mybir.AluOpType.add
    scale = DH ** -0.5

    xs = ext["xs"]; wq_s = ext["wq_s"]; wk_s = ext["wk_s"]; wv_s = ext["wv_s"]
    wp_s = ext["wp_s"]; bqs = ext["bqs"]; bks = ext["bks"]; bvs = ext["bvs"]
    pbs = ext["pbs"]; tris = ext["tris"]
    out_ext = ext["out"]
    vts = ext["vts"]

    qts, kts = [], []
    for n in range(4):
        qts.append(big.tile([128, 4, 512], bf16, tag=f"qT{n}", name=f"qT{n}"))
        kts.append(big.tile([128, 4, 512], bf16, tag=f"kT{n}", name=f"kT{n}"))
    pdt = bf16 if ext.get("proj_bf16") else f32
    proj_dram = dram.tile([T, C], pdt, tag="proj", name="proj")
    rs_out = dram.tile([THALF, C], pdt, tag="rs", name="rs")

    def qkv_chunk_units(n):
        units = []
        for m in range(4):
            for w_s, dsts, bias in ((wq_s, qts, bqs), (wk_s, kts, bks)):
                def u(m=m, w_s=w_s, dsts=dsts, bias=bias, n=n):
                    ps_t = ps2.tile([128, 512], f32, tag="P", name="Pqk")
                    for cc in range(8):
                        nc.tensor.matmul(
                            ps_t[:],
                            lhsT=w_s[:, cc, m * 128:(m + 1) * 128],
                            rhs=xs[:, cc, n * 512:(n + 1) * 512],
                            start=(cc == 0), stop=(cc == 7),
                        )
                    nc.vector.tensor_tensor(
                        dsts[n][:, m, :], ps_t[:],
                        bias[:, m:m + 1].to_broadcast([128, 512]), add)
                units.append(u)
        for tl in range(4):
            def u(tl=tl, n=n):
                tt = 4 * n + tl
                ps_t = ps2.tile([128, 512], f32, tag="P", name="Pv")
                for cc in range(8):
                    nc.tensor.matmul(
                        ps_t[:],
                        lhsT=xs[:, cc, tt * 128:(tt + 1) * 128],
                        rhs=wv_s[:, cc, :],
                        start=(cc == 0), stop=(cc == 7),
                    )
                nc.vector.tensor_tensor(
                    vts[n][:, :, tl, 0:DH],
                    ps_t[:].rearrange("p (h d) -> p h d", h=HL),
                    bvs[:].rearrange("p (h d) -> p h d", h=HL),
                    add,
                )
            units.append(u)
        return units

    def proj_units(qt, attnT):
        units = []
        for tt in range(4):
            t0 = qt * 512 + tt * 128
            for cn in range(2):
                def u(tt=tt, cn=cn, t0=t0, attnT=attnT):
                    ps_P = ps2.tile([128, 512], f32, tag="P", name="Pp")
                    for jj in range(4):
                        nc.tensor.matmul(
                            ps_P[:],
                            lhsT=attnT[:, jj, tt * 128:(tt + 1) * 128],
                            rhs=wp_s[:, jj, cn * 512:(cn + 1) * 512],
                            start=(jj == 0), stop=(jj == 3),
                        )
                    ot = small.tile([128, 512], pdt, tag="ot", name="ot")
                    nc.vector.tensor_tensor(
                        ot[:], ps_P[:], pbs[:, cn * 512:(cn + 1) * 512], add)
                    nc.sync.dma_start(
                        out=proj_dram[t0:t0 + 128, cn * 512:(cn + 1) * 512],
                        in_=ot[:])
                units.append(u)
        return units

    def rs_unit(qt):
        def u(qt=qt):
            if ext.get("no_rs"):
                nc.sync.dma_start(
                    out=rs_out[qt * 256:(qt + 1) * 256, :],
                    in_=proj_dram[qt * 512:qt * 512 + 256, :])
            else:
                nc.gpsimd.collective_compute(
                    "ReduceScatter",
                    mybir.AluOpType.add,
                    replica_groups=REPLICA_GROUPS,
                    ins=[proj_dram[qt * 512:(qt + 1) * 512, :].opt()],
                    outs=[rs_out[qt * 256:(qt + 1) * 256, :].opt()],
                )
                if ext.get("proj_bf16"):
                    for hh in range(2):
                        r0 = qt * 256 + hh * 128
                        stg = small.tile([128, C], bf16, tag="stg",
                                         name="stg")
                        nc.sync.dma_start(out=stg[:], in_=rs_out[r0:r0 + 128, :])
                        stf = small.tile([128, C], f32, tag="stf", name="stf")
                        nc.vector.tensor_copy(stf[:], stg[:])
                        nc.sync.dma_start(out=out_ext[r0:r0 + 128, :],
                                          in_=stf[:])
                else:
                    nc.sync.dma_start(
                        out=out_ext[qt * 256:(qt + 1) * 256, :],
                        in_=rs_out[qt * 256:(qt + 1) * 256, :])
        return [u]

    def attn_unit(qt, j, attnT):
        # Per-kc processing with ping-pong score tiles: halves side-by-side
        # in one [128,1024] tile (half0 at [0:w], half1 at [512:512+w]), one
        # exp per kc covering both halves, so exp(kc) overlaps S MMs(kc+1).
        # The two S MMs target row-groups (0,·)/(64,·) and run concurrently.
        kmax = 4 * (qt + 1)
        ps_O0 = ps1.tile([128, 512], f32, tag="O0", name="O0")
        ps_O1 = ps1.tile([128, 512], f32, tag="O1", name="O1")

        def width(kc):
            d = kc - 4 * qt
            return 512 if d < 0 else 512 - 128 * d

        def emit_S(kc):
            w = width(kc)
            ps_S = ps1.tile([128, 1024], f32, tag=f"S{kc % 2}",
                            name="ps_S")
            for half, base in ((0, 0), (64, 512)):
                nc.tensor.matmul(
                    ps_S[:, base:base + w],
                    lhsT=kts[kc // 4][half:half + 64, j,
                                      (kc % 4) * 128:(kc % 4 + 1) * 128],
                    rhs=qts[qt][half:half + 64, j, 512 - w:512],
                    start=True, stop=True,
                )
            return ps_S

        # s_ahead=2: PE stream per link is [S(kc+2), AV(kc-2)] — the AV lags
        # the exp that produced its pt by two full links, so the PE never
        # parks on a just-issued exp/mask completion, and exp(kc+1)'s S is
        # already in PSUM when exp(kc) retires -> ACT streams back-to-back.
        # s_ahead=1: [S(kc+2), AV(kc)] (PE waits each fresh exp).
        # s_ahead=0: [S(kc), AV(kc)] after exp(kc) (original chain).
        s_ahead = ext.get("s_ahead", 0)
        no_av = "av" in ext.get("skip", ())
        no_exp = "exp" in ext.get("skip", ())
        av_lag = 2 if s_ahead == 2 else 0

        av64 = ext.get("av64", 1)

        def emit_AV(kc, pt):
            w = width(kc)
            n, lc = kc // 4, kc % 4
            if not av64:
                for hi, (base, ps_O) in enumerate(((0, ps_O0), (512, ps_O1))):
                    nc.tensor.matmul(
                        ps_O[:, 512 - w:512],
                        lhsT=vts[n][:, 2 * j + hi, lc, :],
                        rhs=pt[:, base:base + w],
                        start=(kc == 0), stop=(kc == kmax - 1),
                    )
                return
            # Key-split AV: 4 K=64 matmuls with row-groups alternating
            # r0/r64 so every LDWEIGHTS pulls ahead of the other-half MM
            # in flight, and disjoint-row disjoint-bank pairs overlap.
            # (hi, key-half, dest): O0 and O1 each accumulate both halves.
            seq = ((0, 0, ps_O0), (1, 64, ps_O1), (1, 0, ps_O1),
                   (0, 64, ps_O0))
            for i, (hi, rb, ps_O) in enumerate(seq):
                nc.tensor.matmul(
                    ps_O[:, 512 - w:512],
                    lhsT=vts[n][rb:rb + 64, 2 * j + hi, lc, :],
                    rhs=pt[rb:rb + 64, hi * 512:hi * 512 + w],
                    start=(kc == 0 and i < 2), stop=(kc == kmax - 1 and i >= 2),
                )

        def emit_exp_mask(kc, ps_S):
            w = width(kc)
            pt = pt_pool.tile([128, 1024], bf16, tag="pt", name="pt")
            nc.scalar.activation(pt[:, 0:512 + w], ps_S[:, 0:512 + w], Exp,
                                 scale=scale)
            if kc >= 4 * qt:
                for base in (0, 512):
                    nc.vector.tensor_tensor(
                        pt[:, base:base + 128], pt[:, base:base + 128],
                        tris[:, 0:128], mul)
            return pt

        if s_ahead == 3 and not (no_av or no_exp):
            # Bunched pipeline: per 2-link iteration emit
            #   exp(kc0) exp(kc1) | S(kc0+2) S(kc0+3) | AV(kc0-2) AV(kc0-1)
            # Same-type matmuls stay adjacent so LDWEIGHTS pulls ahead via
            # the background buffer; only two S<->AV boundaries per 2 links
            # pay an exposed weight load. AV lags its exp by a full
            # iteration so the PE never parks on a fresh exp/mask sem.
            ps_Ss = {0: emit_S(0), 1: emit_S(1)}
            pts = {}
            for kc0 in range(0, kmax, 2):
                for kc in (kc0, kc0 + 1):
                    pts[kc] = emit_exp_mask(kc, ps_Ss.pop(kc))
                for kc in (kc0 + 2, kc0 + 3):
                    if kc < kmax:
                        ps_Ss[kc] = emit_S(kc)
                for kc in (kc0 - 2, kc0 - 1):
                    if kc >= 0:
                        emit_AV(kc, pts.pop(kc))
            for kc in sorted(pts):
                emit_AV(kc, pts.pop(kc))
            for half, ps_O in ((0, ps_O0), (64, ps_O1)):
                sums = small.tile([64, 512], f32, tag="rb", name="sums")
                nc.vector.tensor_copy(sums[:], ps_O[64:128, :])
                rb = small.tile([64, 512], f32, tag="rb2", name="rb")
                nc.vector.reciprocal_approx_fast(rb[:], sums[:])
                with nc.allow_low_precision(reason="bf16 norm mult"):
                    nc.vector.tensor_tensor(attnT[half:half + 64, j, :],
                                            ps_O[0:64, :], rb[:], mul)
            return

        ps_Ss = {}
        if s_ahead:
            ps_Ss[0] = emit_S(0)
            if kmax > 1:
                ps_Ss[1] = emit_S(1)
        if no_exp and "pts_static" not in ext:
            # clean PE-side probe: pre-made pt slots, no per-kc writes
            ext["pts_static"] = []
            for i in range(4):
                spt = pt_pool.tile([128, 1024], bf16, tag="pt",
                                   name="spt")
                nc.vector.memset(spt[:], 0.5)
                ext["pts_static"].append(spt)
        pts = {}
        for kc in range(kmax):
            w = width(kc)
            ps_S = ps_Ss.pop(kc) if kc in ps_Ss else emit_S(kc)
            if no_exp:
                pts[kc] = ext["pts_static"][kc % 4]
                if kc - av_lag >= 0:
                    emit_AV(kc - av_lag, pts.pop(kc - av_lag))
                continue
            pt = pt_pool.tile([128, 1024], bf16, tag="pt", name="pt")
            pts[kc] = pt
            if not no_exp:
                # single exp over [0:512+w] spans the [w:512] hole when
                # w<512 (cheaper than a second activation's overhead)
                nc.scalar.activation(pt[:, 0:512 + w], ps_S[:, 0:512 + w],
                                     Exp, scale=scale)
                if kc >= 4 * qt and not no_av:
                    # causal boundary: first 128 cols of the block
                    for base in (0, 512):
                        nc.vector.tensor_tensor(
                            pt[:, base:base + 128], pt[:, base:base + 128],
                            tris[:, 0:128], mul)
            if s_ahead and kc + 2 < kmax:
                ps_Ss[kc + 2] = emit_S(kc + 2)
            if no_av:
                # probe: consume pt cheaply so exps aren't dead code
                pts.pop(kc)
                nc.vector.tensor_tensor(
                    ext["acc"][:], ext["acc"][:], pt[:, 0:8],
                    mybir.AluOpType.max)
                continue
            if no_exp:
                nc.vector.memset(pt[:, 0:8], 1.0)  # allocate pt w/o exp
            if kc - av_lag >= 0:
                emit_AV(kc - av_lag, pts.pop(kc - av_lag))
        if no_av:
            return
        for kc in sorted(pts):
            emit_AV(kc, pts.pop(kc))
        for half, ps_O in ((0, ps_O0), (64, ps_O1)):
            sums = small.tile([64, 512], f32, tag="rb", name="sums")
            nc.vector.tensor_copy(sums[:], ps_O[64:128, :])
            rb = small.tile([64, 512], f32, tag="rb2", name="rb")
            nc.vector.reciprocal_approx_fast(rb[:], sums[:])
            with nc.allow_low_precision(reason="bf16 norm mult"):
                nc.vector.tensor_tensor(attnT[half:half + 64, j, :],
                                        ps_O[0:64, :], rb[:], mul)

    # ---- emission: prologue, then attention rounds with filler ----
    from collections import deque
    skip = ext.get("skip", ())
    if "attn" in skip:
        snk = ext["snk"]
        for n in range(4):
            for u in qkv_chunk_units(n):
                u()
            nc.sync.dma_start(out=snk[0:128, n * 2048:(n + 1) * 2048],
                              in_=qts[n][:].rearrange("p a b -> p (a b)"))
        return
    do_qkv = "qkv" not in skip
    do_proj = "proj" not in skip
    no_av = "av" in skip
    if no_av:
        acc = small.tile([128, 8], bf16, tag="acc", name="acc")
        nc.vector.memset(acc[:], 0.0)
        ext["acc"] = acc
    filler = deque()
    if do_qkv:
        c0 = qkv_chunk_units(0)
        if ext.get("ptrim"):
            # prologue: only what attention unit (0,0) needs — Q/K slab m=0
            # and the V chains; the rest fills round 0
            for u in c0[0:2] + c0[8:12]:
                u()
            filler.extend(c0[2:8])
        else:
            for u in c0:
                u()
        filler.extend(qkv_chunk_units(1))
    else:
        # timing-only variant: touch q/k tiles so Tile allocates them
        for n in range(4):
            nc.vector.memset(qts[n][:, :, 0:1], 1.0)
            nc.vector.memset(kts[n][:, :, 0:1], 1.0)
    attnTs = {}
    for qt in range(4):
        attnT = sb.tile([128, 4, 512], bf16, tag="attnT", name="attnT")
        for j in range(4):
            attn_unit(qt, j, attnT)
            npop = (len(filler) + (3 - j)) // (4 - j)
            for _ in range(npop):
                filler.popleft()()
        attnTs[qt] = attnT
        if do_qkv:
            if qt == 0:
                filler.extend(qkv_chunk_units(2))
            elif qt == 1:
                filler.extend(qkv_chunk_units(3))
        if not do_proj:
            snk = ext["snk"]
            if no_av:
                nc.sync.dma_start(out=snk[qt:qt + 1, 0:8],
                                  in_=ext["acc"][0:1, :])
            else:
                nc.sync.dma_start(out=snk[qt * 128:(qt + 1) * 128, 0:2048],
                                  in_=attnT[:].rearrange("p a b -> p (a b)"))
            continue
        if qt == 1:
            filler.extend(proj_units(0, attnTs[0]))
            filler.extend(rs_unit(0))
        elif qt == 2:
            filler.extend(proj_units(1, attnTs[1]))
            filler.extend(rs_unit(1))
            filler.extend(proj_units(2, attnTs[2]))
            filler.extend(rs_unit(2))
    while filler:
        filler.popleft()()
    if do_proj:
        for u in proj_units(3, attnTs[3]) + rs_unit(3):
            u()


def build_graph(reps=1, single_core=False, no_rs=False, skip=(),
                norm_dma=False, wide_exp=False, loop_n=0,
                pt_bufs=4, sb_bufs=2, small_bufs=3, swap_side=False,
                proj_bf16=True, defer_k=1, qk1024=False, narrowmask=False,
                ldwshare=False, stagger=False, body_reps=1, v2=True,
                s_ahead=2, av64=0, ptrim=0):
    nc = bacc.Bacc("TRN2", target_bir_lowering=False, debug=False,
                   num_devices=1 if single_core else 8)
    xT_e = nc.dram_tensor("xT", [C, T], bf16, kind="ExternalInput").ap()
    wq_e = nc.dram_tensor("wq", [C, CL], bf16, kind="ExternalInput").ap()
    wk_e = nc.dram_tensor("wk", [C, CL], bf16, kind="ExternalInput").ap()
    wv_e = nc.dram_tensor("wv", [C, CL], bf16, kind="ExternalInput").ap()
    wp_e = nc.dram_tensor("wp", [CL, C], bf16, kind="ExternalInput").ap()
    bq_e = nc.dram_tensor("bq", [128, 4], f32, kind="ExternalInput").ap()
    bk_e = nc.dram_tensor("bk", [128, 4], f32, kind="ExternalInput").ap()
    bv_e = nc.dram_tensor("bv", [1, CL], f32, kind="ExternalInput").ap()
    pb_e = nc.dram_tensor("pb", [1, C], f32, kind="ExternalInput").ap()
    tri_e = nc.dram_tensor("tri", [128, 2048], bf16, kind="ExternalInput").ap()
    out_e = nc.dram_tensor("out", [THALF, C], f32, kind="ExternalOutput").ap()
    snk_e = (nc.dram_tensor("snk", [512, 8320], bf16, kind="ExternalOutput").ap()
             if skip else None)

    if v2:
        sb_bufs = max(sb_bufs, 4)   # attnT read by proj up to 2 rounds later
        pt_bufs = max(pt_bufs, 4)   # pt must outlive AV lag 2
    with tile.TileContext(nc) as tc:
        if swap_side:
            tc.swap_default_side()
        with (
            tc.tile_pool(name="const", bufs=1) as const,
            tc.tile_pool(name="big", bufs=1) as big,
            tc.tile_pool(name="sb", bufs=sb_bufs) as sb,
            tc.tile_pool(name="pt", bufs=pt_bufs) as pt_pool,
            tc.tile_pool(name="small", bufs=small_bufs) as small,
            tc.tile_pool(name="ps1", bufs=1, space="PSUM") as ps1,
            tc.tile_pool(name="ps2", bufs=2, space="PSUM") as ps2,
            tc.tile_pool(name="dram", bufs=2, space="DRAM") as dram,
        ):
            ps = ps1 if v2 else ps2  # v1 keeps its bufs=2 pool as "ps"
            # load constants once
            xs = const.tile([128, 8, T], bf16, tag="xs")
            for cc in range(8):
                nc.sync.dma_start(
                    out=xs[:, cc, :],
                    in_=xT_e.rearrange("(c p) t -> p c t", p=128)[:, cc, :])
            wq_s = const.tile([128, 8, CL], bf16, tag="wq")
            wk_s = const.tile([128, 8, CL], bf16, tag="wk")
            wv_s = const.tile([128, 8, CL], bf16, tag="wv")
            for w_s, w_e in ((wq_s, wq_e), (wk_s, wk_e), (wv_s, wv_e)):
                for cc in range(8):
                    nc.sync.dma_start(
                        out=w_s[:, cc, :],
                        in_=w_e.rearrange("(c p) n -> p c n", p=128)[:, cc, :])
            wp_s = const.tile([128, 4, C], bf16, tag="wp")
            for cc in range(4):
                nc.sync.dma_start(
                    out=wp_s[:, cc, :],
                    in_=wp_e.rearrange("(c p) n -> p c n", p=128)[:, cc, :])
            bqs = const.tile([128, 4], f32, tag="bq")
            nc.sync.dma_start(out=bqs[:], in_=bq_e)
            bks = const.tile([128, 4], f32, tag="bk")
            nc.sync.dma_start(out=bks[:], in_=bk_e)
            bvs = const.tile([128, CL], f32, tag="bv")
            nc.sync.dma_start(out=bvs[:], in_=bv_e.to_broadcast([128, CL]))
            pbs = const.tile([128, C], f32, tag="pb")
            nc.sync.dma_start(out=pbs[:], in_=pb_e.to_broadcast([128, C]))
            tris = const.tile([128, 2048], bf16, tag="tri")
            nc.sync.dma_start(out=tris[:], in_=tri_e)
            vts = []
            for n in range(4):
                vts.append(big.tile([128, HL, 4, 128], bf16, tag=f"v{n}",
                                    name=f"v{n}"))
                nc.vector.memset(vts[n][:, :, :, DH:128], 1.0)

            ext = dict(xs=xs, wq_s=wq_s, wk_s=wk_s, wv_s=wv_s, wp_s=wp_s,
                       bqs=bqs, bks=bks, bvs=bvs, pbs=pbs, tris=tris,
                       vts=vts, out=out_e, snk=snk_e,
                       proj_bf16=proj_bf16,
                       single_core=single_core, no_rs=no_rs, skip=skip,
                       norm_dma=norm_dma, wide_exp=wide_exp,
                       defer_k=defer_k, qk1024=qk1024,
                       narrowmask=narrowmask, ldwshare=ldwshare,
                       s_ahead=s_ahead, av64=av64, ptrim=ptrim)
            pools = dict(sb=sb, big=big, ps=ps, ps1=ps1, ps2=ps2, pt=pt_pool,
                         small=small, dram=dram)
            body = build_body_v2 if v2 else build_body
            if loop_n:
                hints = (mybir.EngineType.PE, mybir.EngineType.DVE,
                         mybir.EngineType.Activation, mybir.EngineType.SP,
                         mybir.EngineType.Pool)
                with tc.For_i(0, loop_n, 1, hint_engines=hints,
                              staggered_reset=stagger):
                    for _r in range(body_reps):
                        body(nc, tc, ext, pools)
            else:
                for r in range(reps):
                    body(nc, tc, ext, pools)

    nc.compile()
    return nc


def prep_shards(x, qkv_w, qkv_b, proj_w, proj_b):
    """Host-side sharding + layout prep. Returns in_maps for 8 cores."""
    kr = np.arange(128)[:, None]
    qr = np.arange(512)[None, :]
    tri1 = (qr >= kr).astype(np.float32)          # canonical triangle [128,512]
    pad = np.ones((128, 1), np.float32)
    # packed per-diagonal-pair masks matching the contiguous S layout:
    # dp0 widths (512, 384), dp1 widths (256, 128); rest padded with 1.0
    trip0 = np.concatenate(
        [tri1, tri1[:, 0:384], np.repeat(pad, 128, 1)], axis=1)
    trip1 = np.concatenate(
        [tri1[:, 0:256], tri1[:, 0:128], np.repeat(pad, 640, 1)], axis=1)
    tri = np.concatenate([trip0, trip1], axis=1).astype(BF)
    x = np.asarray(x, np.float32)
    qkv_w = np.asarray(qkv_w, np.float32)
    qkv_b = np.asarray(qkv_b, np.float32)
    proj_w = np.asarray(proj_w, np.float32)
    proj_b = np.asarray(proj_b, np.float32)

    in_maps = []
    for core in range(8):
        b, g = core // 2, core % 2
        hsl = slice(g * CL, (g + 1) * CL)
        wq = qkv_w[0 * C:1 * C][hsl]
        wk = qkv_w[1 * C:2 * C][hsl]
        wv = qkv_w[2 * C:3 * C][hsl]
        in_maps.append({
            "xT": np.ascontiguousarray(x[b].T).astype(BF),
            "wq": np.ascontiguousarray(wq.T).astype(BF),
            "wk": np.ascontiguousarray(wk.T).astype(BF),
            "wv": np.ascontiguousarray(wv.T).astype(BF),
            "wp": np.ascontiguousarray(proj_w[:, hsl].T).astype(BF),
            "bq": np.ascontiguousarray(
                qkv_b[0 * C:1 * C][hsl].reshape(4, 128).T).astype(np.float32),
            "bk": np.ascontiguousarray(
                qkv_b[1 * C:2 * C][hsl].reshape(4, 128).T).astype(np.float32),
            "bv": qkv_b[2 * C:3 * C][hsl].reshape(1, CL).astype(np.float32),
            "pb": (proj_b if g == 0 else np.zeros_like(proj_b)
                   ).reshape(1, C).astype(np.float32),
            "tri": tri,
        })
    return in_maps


def assemble(results):
    # chunked ReduceScatter: per q-tile chunk of 512 rows, rank 0 holds the
    # first 256 reduced rows, rank 1 the last 256
    out = np.empty((B, T, C), np.float32)
    for b in range(B):
        lo = results[2 * b]["out"]
        hi = results[2 * b + 1]["out"]
        for qt in range(4):
            out[b, qt * 512:qt * 512 + 256] = lo[qt * 256:(qt + 1) * 256]
            out[b, qt * 512 + 256:(qt + 1) * 512] = hi[qt * 256:(qt + 1) * 256]
    return out


_CACHE = {}


def _numpy_fallback(x, qkv_w, qkv_b, proj_w, proj_b, mask):
    x = np.asarray(x, np.float32)
    qkv = x @ np.asarray(qkv_w, np.float32).T + np.asarray(qkv_b, np.float32)
    qkv = qkv.reshape(B, T, 3, H, DH).transpose(2, 0, 3, 1, 4)
    q, k, v = qkv[0], qkv[1], qkv[2]
    att = np.einsum("bhqd,bhkd->bhqk", q, k) * (DH ** -0.5)
    att = np.where(np.asarray(mask), att, -np.inf)
    att = att - att.max(axis=-1, keepdims=True)
    att = np.exp(att)
    att /= att.sum(axis=-1, keepdims=True)
    o = np.einsum("bhqk,bhkd->bhqd", att, v)
    o = o.transpose(0, 2, 1, 3).reshape(B, T, C)
    return (o @ np.asarray(proj_w, np.float32).T
            + np.asarray(proj_b, np.float32)).astype(np.float32)


def kernel(x, qkv_w, qkv_b, proj_w, proj_b, mask):
    causal = np.tril(np.ones((T, T), dtype=bool))
    if not np.array_equal(np.asarray(mask).reshape(T, T), causal):
        return _numpy_fallback(x, qkv_w, qkv_b, proj_w, proj_b, mask)

    if "nc" not in _CACHE:
        _CACHE["nc"] = build_graph(reps=1)
    nc = _CACHE["nc"]
    in_maps = prep_shards(x, qkv_w, qkv_b, proj_w, proj_b)
    res = run_bass_kernel_spmd(nc, in_maps, core_ids=list(range(8)))
    return assemble(res.results)

